# revision 24
# baseline (speedup 1.0000x reference)
"""Trainium2 Bass kernel for nn_G3DCrossAttention (B=2, C=512, L=2048, G=2048, H=8).

Algebraic structure (exact math): exp_p[g,b,:] = exp[b,g]*Wg[:,0]+bg is rank-1, so
k/v collapse to k = e*u_k + c_k, v = e*u_v + c_v.  The j-constant score shift
cancels in softmax, the attention output collapses per head to
    x_attn = w*u_v + c_v,   w_i = f_b(a_i),  a = x_seq @ M + a0,
with f_b(a) = d/da log Z_b(a),  Z_b(a) = sum_j exp(a*e_bj).  On device, log Z is
sampled at 32 Chebyshev nodes (exp + accum), and a host-precomputed linear map
(fit + analytic series derivative) turns those samples into degree-20 Chebyshev
coefficients of f_b.  f is evaluated at all (i,h) via a T_k recurrence in a
packed [128,32] layout, unpacked to [H,T] by one SBUF->SBUF DMA, and applied as
one outer-product matmul per 128-channel tile.

All weight-only transforms (u_k/u_v/c_v, M, a0, LN2 folded into Wo'=Wo*g2,
bo'=bo+Wo@be2, s2=Wo'@1) are computed on HOST; the device sees three fp16
weight mats (W1.T, W2.T, Wo'.T), the f32 seq slice, exp, and one packed
[128,742]+[1,2176] constant grid.  LN2's normalization is folded into the
output projection: out = rstd2 .* (Wo'@y2 - s2(x)mu2 + bo'(x)std2).

Sharding: data-parallel over L (LC=256 queries/core), full pipeline per core.
"""

from contextlib import ExitStack

import numpy as np

import concourse.bass as bass
import concourse.tile as tile
from concourse import bacc, mybir
from concourse.bass_utils import run_bass_kernel_spmd

F32 = mybir.dt.float32
F32R = mybir.dt.float32r
FP16 = mybir.dt.float16
AF = mybir.ActivationFunctionType
OP = mybir.AluOpType

B, C, L, G, H = 2, 512, 2048, 2048, 8
D = C // H
NCORES = 8
LC = L // NCORES              # 256 queries per core
T = B * LC                    # 512 tokens per core, tau = b*LC + l
KC = C // 128                 # 4
KH = (4 * C) // 128           # 16
FP = 32                       # llo width of the packed a/w layout
NLHI = LC // FP               # 8
SCALE = 1.0 / float(np.sqrt(D))
EPS = 1e-5
SCAL = 5.0                    # Chebyshev half-range in a units (|a|max ~ 4.43)
KD = 16                       # Chebyshev series length for f = (logZ)'
MN = 32                       # logZ sample nodes per batch
NWARM = 7                     # PE warm-up matmuls while DMAs land

# ---- smalls grid column layout (f32 [128, SM_NCOL]) -------------------------
SM_M = 0                      # [128, 32]  M' tiles (kt-major, 8 cols each)
SM_UVH = 32                   # [8, 512]   u_v gathered into head rows
SM_DCT = 544                  # [32, KD]   logZ samples -> f coeffs
SM_ID = 564                   # [KD, KD]   identity for PE transpose
SM_SEL = 584                  # [2, 128]   batch selector for coeff broadcast
SM_XN = 712                   # [32, 1]    a-space Chebyshev nodes
SM_A0 = 713                   # [8, 1]     a0' bias
SM_CV = 714                   # [128, 4]   c_v per kt tile
SM_B1 = 718                   # [128, 16]  b1 per mt tile
SM_B2 = 734                   # [128, 4]
SM_BE1 = 738                  # [128, 4]
SM_NCOL = 742

# ---- rows vector layout (f32 [1, RW_NCOL]) ----------------------------------
RW_G1 = 0                     # g1 [C]
RW_NS2 = 512                  # -s2 [C]
RW_ONE = 1024                 # ones [512]
RW_BO = 1536                  # bo' [C]
RW_A0R = 2048                 # a0' [8]
RW_NCOL = 2056

TRACE = False
TRACE_KW = {}
LAST_RESULTS = None
_CACHE = None
DBG = False


def _host_consts():
    """Input-independent matrices for the smalls grid."""
    m = np.arange(MN)
    theta = np.pi * (2 * m + 1) / (2 * MN)
    xn = (SCAL * np.cos(theta)).astype(np.float32)          # nodes in a units
    F = np.zeros((KD, MN))
    for k in range(KD):
        F[k] = (2.0 / MN) * np.cos(k * theta)
    F[0] *= 0.5
    import numpy.polynomial.chebyshev as Ch
    DER = np.zeros((KD, KD))
    for k in range(KD):
        ck = np.zeros(KD)
        ck[k] = 1
        dd = Ch.chebder(ck)
        DER[:len(dd), k] = dd
    DM = (DER @ F) / SCAL                                   # [KD, MN]
    return xn, DM.T.astype(np.float32)                      # dct1 [MN, KD]


_XN, _DCT1 = _host_consts()


def _build():
    nc = bacc.Bacc(debug=False, num_devices=NCORES)

    seq_sl = nc.dram_tensor("seq_sl", [B, C, LC], F32, kind="ExternalInput")
    expv = nc.dram_tensor("expv", [B, G], F32, kind="ExternalInput")
    smalls = nc.dram_tensor("smalls", [128, SM_NCOL], F32, kind="ExternalInput")
    rowsv = nc.dram_tensor("rowsv", [1, RW_NCOL], F32, kind="ExternalInput")
    w1a = nc.dram_tensor("w1a", [C, 4 * C], FP16, kind="ExternalInput")   # W1.T
    w2a = nc.dram_tensor("w2a", [4 * C, C], FP16, kind="ExternalInput")   # W2.T
    woa = nc.dram_tensor("woa", [C, C], FP16, kind="ExternalInput")       # Wo'.T
    out_sl = nc.dram_tensor("out_sl", [B, C, LC], F32, kind="ExternalOutput")
    dbg = {}
    if DBG:
        dbg["tt_sb"] = nc.dram_tensor("d_ttsb", [8, T], F32, kind="ExternalOutput")
        dbg["tt"] = nc.dram_tensor("d_tt", [128, FP], F32, kind="ExternalOutput")
        dbg["lnz"] = nc.dram_tensor("d_lnz", [MN, B], F32, kind="ExternalOutput")
        dbg["cbb"] = nc.dram_tensor("d_cbb", [128, KD], F32, kind="ExternalOutput")
        dbg["wp"] = nc.dram_tensor("d_wp", [128, FP], F32, kind="ExternalOutput")
        dbg["wH"] = nc.dram_tensor("d_wH", [H, T], F32, kind="ExternalOutput")
        dbg["y"] = nc.dram_tensor("d_y", [KC, 128, T], FP16, kind="ExternalOutput")
        dbg["x"] = nc.dram_tensor("d_x", [KC, 128, T], FP16, kind="ExternalOutput")
        dbg["h0"] = nc.dram_tensor("d_h0", [128, T], FP16, kind="ExternalOutput")
        dbg["y2"] = nc.dram_tensor("d_y2", [KC, 128, T], FP16, kind="ExternalOutput")
        dbg["rstd1"] = nc.dram_tensor("d_rstd1", [1, T], F32, kind="ExternalOutput")

    with tile.TileContext(nc) as tc, ExitStack() as ctx:
        p_w = ctx.enter_context(tc.tile_pool(name="w", bufs=1))
        p_act = ctx.enter_context(tc.tile_pool(name="act", bufs=1))
        p_sm = ctx.enter_context(tc.tile_pool(name="sm", bufs=1))
        ps_mm = ctx.enter_context(tc.tile_pool(name="psmm", bufs=2, space="PSUM"))
        ps_xa = ctx.enter_context(tc.tile_pool(name="psxa", bufs=2, space="PSUM"))
        ps_st = ctx.enter_context(tc.tile_pool(name="psst", bufs=1, space="PSUM"))
        ps_pa = ctx.enter_context(tc.tile_pool(name="pspa", bufs=1, space="PSUM"))
        ps_ck = ctx.enter_context(tc.tile_pool(name="psck", bufs=1, space="PSUM"))

        # ---- tiny on-chip constants (no DMA) -----------------------------
        wtile_f = p_sm.tile([128, T], F32, tag="warmf")
        nc.vector.memset(wtile_f[:], 0.0)
        wtile = p_sm.tile([128, T], F32R, tag="warm")
        nc.vector.tensor_copy(wtile[:], wtile_f[:])
        onesk = p_sm.tile([128, 1], FP16, tag="onesk")
        nc.vector.memset(onesk[:], 1.0 / C)
        eps_col = p_sm.tile([1, 1], F32, tag="epsc")
        nc.vector.memset(eps_col[:], EPS)

        # ---- DMA loads: 5 independent queue rows -------------------------
        sm = p_sm.tile([128, SM_NCOL], F32, tag="sm")
        nc.scalar.dma_start(sm[:], smalls[:])
        evb = p_sm.tile([1, B * G], F32, tag="evb")
        nc.scalar.dma_start(evb[:], expv[:])
        eb = p_act.tile([MN, B * G], F32, tag="eb")
        for b in range(B):
            nc.gpsimd.partition_broadcast(eb[0:MN, b * G:(b + 1) * G],
                                          evb[0:1, b * G:(b + 1) * G])

        rows = p_sm.tile([1, RW_NCOL], F32, tag="rows")
        nc.scalar.dma_start(rows[:], rowsv[:])
        xs = p_w.tile([128, KC, B, LC], F32R, tag="xs")
        for b in range(B):
            nc.sync.dma_start(xs[:, :, b, :],
                              seq_sl[b].rearrange("(kt p) l -> p kt l", p=128).bitcast(F32R))
        w1s = p_w.tile([128, KC, 4 * C], FP16, tag="w1")
        nc.sync.dma_start(w1s[:], w1a.rearrange("(kt p) m -> p kt m", p=128))

        w2s = p_w.tile([128, KH, C], FP16, tag="w2")
        nc.gpsimd.dma_start(w2s[:], w2a.rearrange("(kh p) m -> p kh m", p=128))
        wos = p_w.tile([128, KC, C], FP16, tag="wo")
        nc.gpsimd.dma_start(wos[:], woa.rearrange("(kt p) m -> p kt m", p=128))

        # ---- rounded f32r views of small matmul operands -----------------
        m4r = p_sm.tile([128, KC * 8], F32R, tag="m4r")
        nc.vector.tensor_copy(m4r[:], sm[:, SM_M:SM_M + KC * 8])
        rowsr = p_sm.tile([1, RW_NCOL], F32R, tag="rowsr")
        nc.vector.tensor_copy(rowsr[:], rows[:])
        uvhr = p_sm.tile([H, C], F32R, tag="uvhr")
        nc.vector.tensor_copy(uvhr[:], sm[0:H, SM_UVH:SM_UVH + C])

        # ---- PE warm-up while DMAs land ----------------------------------
        for i in range(NWARM):
            pw = ps_pa.tile([8, T], F32, tag="pa", name=f"warm{i}")
            nc.tensor.matmul(pw[:], wtile[:, 0:8], wtile[:], start=True, stop=True)

        # ---- a = x_seq @ M' + a0'  (pre-scaled to t units) ---------------
        pa = ps_ck.tile([8, T], F32, tag="ck", name="pa")
        for kt in range(KC):
            nc.tensor.matmul(pa[:], m4r[:, kt * 8:(kt + 1) * 8],
                             xs[:, kt, :, :],
                             start=(kt == 0), stop=False)
        nc.tensor.matmul(pa[:], rowsr[0:1, RW_A0R:RW_A0R + 8],
                         rowsr[0:1, RW_ONE:RW_ONE + T], start=False, stop=True)
        tt_sb = p_sm.tile([8, T], F32, tag="tts")
        nc.scalar.copy(tt_sb[:], pa[:])

        # ---- logZ sampling at 32 nodes, both batches ---------------------
        z2 = p_sm.tile([MN, B], F32, tag="z2")
        for b in range(B):
            pn = p_act.tile([MN, G], F32, tag="pn", bufs=2, name=f"pn{b}")
            nc.scalar.activation(pn[:], eb[:, b * G:(b + 1) * G], AF.Exp,
                                 scale=sm[0:MN, SM_XN:SM_XN + 1],
                                 accum_out=z2[:, b:b + 1])
        lnz = p_sm.tile([MN, B], F32, tag="lnz")
        nc.scalar.activation(lnz[:], z2[:], AF.Ln)
        sqpre = p_sm.tile([1, 1], F32, tag="sqpre")
        nc.scalar.activation(sqpre[:], eps_col[:], AF.Sqrt, bias=eps_col[:])

        if DBG:
            nc.gpsimd.dma_start(dbg["tt_sb"][:], tt_sb[:])
            nc.gpsimd.dma_start(dbg["lnz"][:], lnz[:])
        # repack to [128, 32], p = b*64 + h*8 + lhi, free = llo (l=lhi*32+llo)
        # (scalar-queue order: after the exp/lnz chain so it doesn't stall it)
        tt = p_sm.tile([128, FP], F32, tag="tt")
        for b in range(B):
            nc.scalar.dma_start(tt[b * 64:(b + 1) * 64, :],
                                tt_sb[:, b * LC:(b + 1) * LC])
        nc.vector.tensor_scalar_max(tt[:], tt[:], -1.0)
        nc.vector.tensor_scalar_min(tt[:], tt[:], 1.0)

        # coeffs: ck2 = dct1.T @ lnz [KD, B]; transpose; broadcast to [128, KD]
        ck2_ps = ps_ck.tile([KD, B], F32, tag="ck")
        nc.tensor.matmul(ck2_ps[:], sm[0:MN, SM_DCT:SM_DCT + KD], lnz[:],
                         start=True, stop=True)
        ck2_sb = p_sm.tile([KD, B], F32, tag="ck2s")
        nc.scalar.copy(ck2_sb[:], ck2_ps[:])
        ckT_ps = ps_ck.tile([B, KD], F32, tag="ck", name="ckT_ps")
        nc.tensor.transpose(ckT_ps[:], ck2_sb[:], sm[0:KD, SM_ID:SM_ID + KD])
        ckT_sb = p_sm.tile([B, KD], F32, tag="ckTs")
        nc.scalar.copy(ckT_sb[:], ckT_ps[:])
        cbb_ps = ps_ck.tile([128, KD], F32, tag="ck", name="cbb_ps")
        nc.tensor.matmul(cbb_ps[:], sm[0:B, SM_SEL:SM_SEL + 128], ckT_sb[:],
                         start=True, stop=True)
        cbb = p_sm.tile([128, KD], F32, tag="cbbs")
        nc.scalar.copy(cbb[:], cbb_ps[:])

        if DBG:
            nc.gpsimd.dma_start(dbg["tt"][:], tt[:])
            nc.gpsimd.dma_start(dbg["cbb"][:], cbb[:])
        # ---- Chebyshev T_k recurrence (vector) ---------------------------
        tt2 = p_sm.tile([128, FP], F32, tag="tt2")
        nc.vector.tensor_add(tt2[:], tt[:], tt[:])
        t_tiles = [None, tt]
        for k in range(2, KD):
            tk = p_sm.tile([128, FP], F32, tag=f"t{k}", name=f"t{k}")
            nc.vector.tensor_mul(tk[:], tt2[:], t_tiles[k - 1][:])
            if k == 2:
                nc.vector.tensor_scalar_sub(tk[:], tk[:], 1.0)   # T0 = 1
            else:
                nc.vector.tensor_sub(tk[:], tk[:], t_tiles[k - 2][:])
            t_tiles.append(tk)
            if k in (5, 8, 11, 14):      # PE keep-warm trickle
                tkr = p_sm.tile([128, 8], F32R, tag="tkr", name=f"tkr{k}")
                nc.gpsimd.tensor_copy(tkr[:], tk[:, 0:8])
                pw = ps_pa.tile([8, T], F32, tag="pa", name=f"trk{k}")
                nc.tensor.matmul(pw[:], tkr[:], wtile[:], start=True, stop=True)
        # t0 term is a constant: handled in the k=1 seed below.

        # ---- contraction sum_k c_k T_k (vector) --------------------------
        accA = p_sm.tile([128, FP], F32, tag="accA")
        accB = p_sm.tile([128, FP], F32, tag="accB")
        nc.vector.tensor_scalar(accA[:], tt[:], cbb[:, 1:2], cbb[:, 0:1],
                                op0=OP.mult, op1=OP.add)
        wp_t = p_sm.tile([128, FP], F32, tag="wp", name="wp")
        cur, nxt = accA, accB
        for k in range(2, KD):
            dst = wp_t if k == KD - 1 else nxt
            nc.vector.scalar_tensor_tensor(
                out=dst[:], in0=t_tiles[k][:], scalar=cbb[:, k:k + 1],
                in1=cur[:], op0=OP.mult, op1=OP.add)
            cur, nxt = dst, cur
        w_pack = cur

        if DBG:
            nc.gpsimd.dma_start(dbg["wp"][:], w_pack[:])
        def trickle(dep, nm):
            tkr = p_sm.tile([128, 8], F32R, tag="tkr", name=f"tkr{nm}")
            nc.gpsimd.tensor_copy(tkr[:], dep[:, 0:8])
            pw = ps_pa.tile([8, T], F32, tag="pa", name=f"trw{nm}")
            nc.tensor.matmul(pw[:], tkr[:], wtile[:], start=True, stop=True)

        def ln_stats_tile(st2, y_tile, kt, ph):
            """Mean contribution inline; squares on gpsimd for a deferred pass."""
            stA, stB, sqs = st2
            nc.tensor.matmul(stA[:], onesk[:], y_tile[:],
                             start=(kt == 0), stop=(kt == KC - 1))
            sq = p_act.tile([128, T], FP16, tag="sq", bufs=4, name=f"sq{ph}{kt}")
            nc.scalar.activation(sq[:], y_tile[:], AF.Square)
            sqs.append(sq)

        def ln_stats_close(st2):
            stA, stB, sqs = st2
            for kt, sq in enumerate(sqs):
                nc.tensor.matmul(stB[:], onesk[:], sq[:],
                                 start=(kt == 0), stop=(kt == KC - 1))

        def ln_stats_open(ph):
            stA = ps_st.tile([1, T], F32, tag="stA", name=f"stA{ph}")
            stB = ps_st.tile([1, T], F32, tag="stB", name=f"stB{ph}")
            return stA, stB, []

        # ---- unpack w to [H, T] and apply: y = w*u_v + c_v + x_seq -------
        wH = p_sm.tile([H, T], F32R, tag="wH")
        for b in range(B):
            nc.scalar.dma_start(wH[:, b * LC:(b + 1) * LC],
                                w_pack[b * 64:(b + 1) * 64, :].bitcast(F32R))
        y_t = []
        st1 = ln_stats_open("a")
        for kt in range(KC):
            xa = ps_xa.tile([128, T], F32, tag="xa", name=f"xa{kt}")
            nc.tensor.matmul(xa[:], uvhr[:, kt * 128:(kt + 1) * 128],
                             wH[:], start=True, stop=True)
            yk = p_act.tile([128, T], FP16, tag="y", bufs=4, name=f"y{kt}")
            eng = nc.vector
            eng.scalar_tensor_tensor(
                out=yk[:], in0=xa[:], scalar=sm[:, SM_CV + kt:SM_CV + kt + 1],
                in1=xs[:, kt, :, :].bitcast(F32), op0=OP.add, op1=OP.add)
            y_t.append(yk)
            ln_stats_tile(st1, yk, kt, "a")
            if kt in (1, 3):
                trickle(yk, f"y{kt}")

        if DBG:
            nc.gpsimd.dma_start(dbg["wH"][:], wH[:].bitcast(F32))
            for kt in range(KC):
                nc.gpsimd.dma_start(dbg["y"][kt], y_t[kt][:])

        def ln_rows(st2, ph, want_mu=False, want_q=False):
            """mean/meansq -> (mu, std, rstd, q=mu*rstd) rows [1, T]."""
            stA, stB = st2[0], st2[1]
            musq = p_sm.tile([1, T], F32, tag="lnr", bufs=6, name=f"musq{ph}")
            nc.scalar.activation(musq[:], stA[:], AF.Square)
            var = p_sm.tile([1, T], F32, tag="lnr", bufs=6, name=f"var{ph}")
            nc.vector.tensor_sub(var[:], stB[:], musq[:])
            std = p_sm.tile([1, T], F32R, tag="lnr", bufs=6, name=f"std{ph}")
            nc.scalar.activation(std[:], var[:], AF.Sqrt, bias=eps_col[:])
            pwln = ps_pa.tile([8, T], F32, tag="pa", name=f"pwln{ph}")
            nc.tensor.matmul(pwln[:], rowsr[0:1, RW_ONE:RW_ONE + 8], std[:],
                             start=True, stop=True)
            rstd_f = p_sm.tile([1, T], F32, tag="rstdf", bufs=2, name=f"rstdf{ph}")
            nc.vector.reciprocal_approx_fast(rstd_f[:], std[:].bitcast(F32))
            rstd = p_sm.tile([1, T], F32R, tag="rstd", bufs=2, name=f"rstd{ph}")
            nc.vector.tensor_copy(rstd[:], rstd_f[:])
            mu = q = None
            if want_mu:
                mu = p_sm.tile([1, T], F32R, tag="mu", bufs=2, name=f"mu{ph}")
                nc.vector.tensor_copy(mu[:], stA[:])
            if want_q:
                q = p_sm.tile([1, T], F32R, tag="q", bufs=2, name=f"q{ph}")
                nc.vector.tensor_mul(q[:], stA[:], rstd_f[:])
            return mu, std, rstd, q

        # ---- LN1 apply -> x ----------------------------------------------
        ln_stats_close(st1)
        _, _, rstd1, q1 = ln_rows(st1, "a", want_q=True)
        x_t = []
        for kt in range(KC):
            sl = slice(RW_G1 + kt * 128, RW_G1 + (kt + 1) * 128)
            pA = ps_mm.tile([128, T], F32, tag="mm", name=f"pA{kt}")
            nc.tensor.matmul(pA[:], rowsr[0:1, sl], rstd1[:],
                             start=True, stop=True)
            pB = ps_mm.tile([128, T], F32, tag="mm", name=f"pB{kt}")
            nc.tensor.matmul(pB[:], rowsr[0:1, sl], q1[:],
                             start=True, stop=True)
            eng = nc.vector
            tx = p_act.tile([128, T], F32, tag="tx", bufs=2, name=f"tx{kt}")
            eng.tensor_mul(tx[:], y_t[kt][:], pA[:])
            xo = p_act.tile([128, T], FP16, tag="x", bufs=4, name=f"x{kt}")
            eng.scalar_tensor_tensor(
                out=xo[:], in0=tx[:], scalar=sm[:, SM_BE1 + kt:SM_BE1 + kt + 1],
                in1=pB[:], op0=OP.add, op1=OP.subtract)
            x_t.append(xo)

        if DBG:
            nc.gpsimd.dma_start(dbg["rstd1"][:], rstd1[:].bitcast(F32))
            for kt in range(KC):
                nc.gpsimd.dma_start(dbg["x"][kt], x_t[kt][:])
        # ---- FFN1: h = relu(W1 @ x + b1) ---------------------------------
        h_t = []
        for mt in range(KH):
            pf = ps_mm.tile([128, T], F32, tag="mm", name=f"pf1{mt}")
            for kt in range(KC):
                nc.tensor.matmul(pf[:], w1s[:, kt, mt * 128:(mt + 1) * 128],
                                 x_t[kt][:], start=(kt == 0), stop=(kt == KC - 1))
            hm = p_act.tile([128, T], FP16, tag="h", bufs=KH, name=f"h{mt}")
            nc.scalar.activation(hm[:], pf[:], AF.Relu,
                                 bias=sm[:, SM_B1 + mt:SM_B1 + mt + 1])
            h_t.append(hm)

        if DBG:
            nc.gpsimd.dma_start(dbg["h0"][:], h_t[0][:])
        # ---- FFN2 + residual -> y2 ---------------------------------------
        y2_t = []
        st2 = ln_stats_open("b")
        for mt in range(KC):
            pf = ps_mm.tile([128, T], F32, tag="mm", name=f"pf2{mt}")
            for kh in range(KH):
                nc.tensor.matmul(pf[:], w2s[:, kh, mt * 128:(mt + 1) * 128],
                                 h_t[kh][:], start=(kh == 0), stop=(kh == KH - 1))
            y2 = p_act.tile([128, T], FP16, tag="y2", bufs=4, name=f"y2{mt}")
            eng = nc.vector
            eng.scalar_tensor_tensor(
                out=y2[:], in0=x_t[mt][:], scalar=sm[:, SM_B2 + mt:SM_B2 + mt + 1],
                in1=pf[:], op0=OP.add, op1=OP.add)
            y2_t.append(y2)
            ln_stats_tile(st2, y2, mt, "b")

        if DBG:
            for mt in range(KC):
                nc.gpsimd.dma_start(dbg["y2"][mt], y2_t[mt][:])
        # ---- LN2 folded into output projection ---------------------------
        ln_stats_close(st2)
        mu2, std2, rstd2, _ = ln_rows(st2, "b", want_mu=True)
        rb_ps = ps_xa.tile([128, T], F32, tag="xa", name="rb")
        nc.tensor.matmul(rb_ps[:], rowsr[0:1, RW_ONE:RW_ONE + 128],
                         rstd2[:], start=True, stop=True)
        rb_sb = p_sm.tile([128, T], F32, tag="rbs")
        nc.vector.tensor_copy(rb_sb[:], rb_ps[:])
        for mt in range(KC):
            po = ps_mm.tile([128, T], F32, tag="mm", name=f"po{mt}")
            for kt in range(KC):
                nc.tensor.matmul(po[:], wos[:, kt, mt * 128:(mt + 1) * 128],
                                 y2_t[kt][:], start=(kt == 0), stop=False)
            nc.tensor.matmul(po[:], rowsr[0:1, RW_NS2 + mt * 128:RW_NS2 + (mt + 1) * 128],
                             mu2[:], start=False, stop=False)
            nc.tensor.matmul(po[:], rowsr[0:1, RW_BO + mt * 128:RW_BO + (mt + 1) * 128],
                             std2[:], start=False, stop=True)
            om = p_act.tile([128, T], F32, tag="om", bufs=2, name=f"om{mt}")
            nc.vector.tensor_mul(om[:], po[:], rb_sb[:])
            seng = nc.sync if mt % 2 == 0 else nc.gpsimd
            seng.dma_start(out_sl[:, mt * 128:(mt + 1) * 128, :].rearrange("b c l -> c b l"),
                           om[:])

    nc.compile()
    return nc


def _host_pack(inputs):
    f32 = lambda x: np.asarray(x, dtype=np.float32)
    Wq, Wk, Wv, Wo = (f32(inputs[k]) for k in ("Wq", "Wk", "Wv", "Wo"))
    W1, W2 = f32(inputs["W1"]), f32(inputs["W2"])
    Wg = f32(inputs["Wg"])[:, 0]
    bg, bq, bv, b1, b2, bo = (f32(inputs[k]) for k in ("bg", "bq", "bv", "b1", "b2", "bo"))
    g1, be1, g2, be2 = (f32(inputs[k]) for k in ("g1", "beta1", "g2", "beta2"))

    u_k = Wk @ Wg
    u_v = Wv @ Wg
    c_v = Wv @ bg + bv
    M = np.zeros((C, H), np.float32)
    a0 = np.zeros(H, np.float32)
    for h in range(H):
        ukh = u_k[h * D:(h + 1) * D]
        M[:, h] = Wq[h * D:(h + 1) * D, :].T @ ukh
        a0[h] = bq[h * D:(h + 1) * D] @ ukh
    Mp = M * (SCALE / SCAL)
    a0p = a0 * (SCALE / SCAL)
    uvH = np.zeros((H, C), np.float32)
    for h in range(H):
        uvH[h, h * D:(h + 1) * D] = u_v[h * D:(h + 1) * D]
    Wop = Wo * g2[None, :]
    bop = bo + Wo @ be2
    s2 = Wop.sum(1)

    smalls = np.zeros((128, SM_NCOL), np.float32)
    for kt in range(KC):
        smalls[:, SM_M + kt * 8:SM_M + (kt + 1) * 8] = Mp[kt * 128:(kt + 1) * 128, :]
        smalls[:, SM_CV + kt] = c_v[kt * 128:(kt + 1) * 128]
        smalls[:, SM_B2 + kt] = b2[kt * 128:(kt + 1) * 128]
        smalls[:, SM_BE1 + kt] = be1[kt * 128:(kt + 1) * 128]
    smalls[0:H, SM_UVH:SM_UVH + C] = uvH
    smalls[0:MN, SM_DCT:SM_DCT + KD] = _DCT1
    smalls[0:KD, SM_ID:SM_ID + KD] = np.eye(KD, dtype=np.float32)
    for p in range(128):
        smalls[p // 64, SM_SEL + p] = 1.0
    smalls[0:MN, SM_XN] = _XN
    smalls[0:H, SM_A0] = a0p
    for mt in range(KH):
        smalls[:, SM_B1 + mt] = b1[mt * 128:(mt + 1) * 128]

    rowsv = np.zeros((1, RW_NCOL), np.float32)
    rowsv[0, RW_G1:RW_G1 + C] = g1
    rowsv[0, RW_NS2:RW_NS2 + C] = -s2
    rowsv[0, RW_ONE:RW_ONE + 512] = 1.0
    rowsv[0, RW_BO:RW_BO + C] = bop
    rowsv[0, RW_A0R:RW_A0R + H] = a0p

    f16t = lambda x: np.ascontiguousarray(x.T, dtype=np.float16)
    return {
        "expv": f32(inputs["exp"]),
        "smalls": smalls,
        "rowsv": rowsv,
        "w1a": f16t(W1),
        "w2a": f16t(W2),
        "woa": f16t(Wop),
    }


def kernel(**inputs):
    global _CACHE, LAST_RESULTS
    if _CACHE is None:
        _CACHE = _build()
    nc = _CACHE

    base = _host_pack(inputs)
    seq = np.asarray(inputs["seq"], dtype=np.float32)
    in_maps = []
    for c in range(NCORES):
        m = dict(base)
        m["seq_sl"] = np.ascontiguousarray(seq[:, :, c * LC:(c + 1) * LC])
        in_maps.append(m)

    res = run_bass_kernel_spmd(nc, in_maps, list(range(NCORES)), trace=TRACE,
                               **TRACE_KW)
    LAST_RESULTS = res
    out = np.empty((B, C, L), np.float32)
    for c in range(NCORES):
        out[:, :, c * LC:(c + 1) * LC] = res.results[c]["out_sl"]
    return out


# revision 25
# speedup vs baseline: 1.0063x; 1.0063x over previous
"""Trainium2 Bass kernel for nn_G3DCrossAttention (B=2, C=512, L=2048, G=2048, H=8).

Algebraic structure (exact math): exp_p[g,b,:] = exp[b,g]*Wg[:,0]+bg is rank-1, so
k/v collapse to k = e*u_k + c_k, v = e*u_v + c_v.  The j-constant score shift
cancels in softmax, the attention output collapses per head to
    x_attn = w*u_v + c_v,   w_i = f_b(a_i),  a = x_seq @ M + a0,
with f_b(a) = d/da log Z_b(a),  Z_b(a) = sum_j exp(a*e_bj).  On device, log Z is
sampled at 32 Chebyshev nodes (exp + accum), and a host-precomputed linear map
(fit + analytic series derivative) turns those samples into degree-20 Chebyshev
coefficients of f_b.  f is evaluated at all (i,h) via a T_k recurrence in a
packed [128,32] layout, unpacked to [H,T] by one SBUF->SBUF DMA, and applied as
one outer-product matmul per 128-channel tile.

All weight-only transforms (u_k/u_v/c_v, M, a0, LN2 folded into Wo'=Wo*g2,
bo'=bo+Wo@be2, s2=Wo'@1) are computed on HOST; the device sees three fp16
weight mats (W1.T, W2.T, Wo'.T), the f32 seq slice, exp, and one packed
[128,742]+[1,2176] constant grid.  LN2's normalization is folded into the
output projection: out = rstd2 .* (Wo'@y2 - s2(x)mu2 + bo'(x)std2).

Sharding: data-parallel over L (LC=256 queries/core), full pipeline per core.
"""

from contextlib import ExitStack

import numpy as np

import concourse.bass as bass
import concourse.tile as tile
from concourse import bacc, mybir
from concourse.bass_utils import run_bass_kernel_spmd

F32 = mybir.dt.float32
F32R = mybir.dt.float32r
FP16 = mybir.dt.float16
AF = mybir.ActivationFunctionType
OP = mybir.AluOpType

B, C, L, G, H = 2, 512, 2048, 2048, 8
D = C // H
NCORES = 8
LC = L // NCORES              # 256 queries per core
T = B * LC                    # 512 tokens per core, tau = b*LC + l
KC = C // 128                 # 4
KH = (4 * C) // 128           # 16
FP = 32                       # llo width of the packed a/w layout
NLHI = LC // FP               # 8
SCALE = 1.0 / float(np.sqrt(D))
EPS = 1e-5
SCAL = 5.0                    # Chebyshev half-range in a units (|a|max ~ 4.43)
KD = 16                       # Chebyshev series length for f = (logZ)'
MN = 32                       # logZ sample nodes per batch
NWARM = 7                     # PE warm-up matmuls while DMAs land

# ---- smalls grid column layout (f32 [128, SM_NCOL]) -------------------------
SM_M = 0                      # [128, 32]  M' tiles (kt-major, 8 cols each)
SM_UVH = 32                   # [8, 512]   u_v gathered into head rows
SM_DCT = 544                  # [32, KD]   logZ samples -> f coeffs
SM_ID = 564                   # [KD, KD]   identity for PE transpose
SM_SEL = 584                  # [2, 128]   batch selector for coeff broadcast
SM_XN = 712                   # [32, 1]    a-space Chebyshev nodes
SM_A0 = 713                   # [8, 1]     a0' bias
SM_CV = 714                   # [128, 4]   c_v per kt tile
SM_B1 = 718                   # [128, 16]  b1 per mt tile
SM_B2 = 734                   # [128, 4]
SM_BE1 = 738                  # [128, 4]
SM_NCOL = 742

# ---- rows vector layout (f32 [1, RW_NCOL]) ----------------------------------
RW_G1 = 0                     # g1 [C]
RW_NS2 = 512                  # -s2 [C]
RW_ONE = 1024                 # ones [512]
RW_BO = 1536                  # bo' [C]
RW_A0R = 2048                 # a0' [8]
RW_NCOL = 2056

TRACE = False
TRACE_KW = {}
LAST_RESULTS = None
_CACHE = None
DBG = False


def _host_consts():
    """Input-independent matrices for the smalls grid."""
    m = np.arange(MN)
    theta = np.pi * (2 * m + 1) / (2 * MN)
    xn = (SCAL * np.cos(theta)).astype(np.float32)          # nodes in a units
    F = np.zeros((KD, MN))
    for k in range(KD):
        F[k] = (2.0 / MN) * np.cos(k * theta)
    F[0] *= 0.5
    import numpy.polynomial.chebyshev as Ch
    DER = np.zeros((KD, KD))
    for k in range(KD):
        ck = np.zeros(KD)
        ck[k] = 1
        dd = Ch.chebder(ck)
        DER[:len(dd), k] = dd
    DM = (DER @ F) / SCAL                                   # [KD, MN]
    return xn, DM.T.astype(np.float32)                      # dct1 [MN, KD]


_XN, _DCT1 = _host_consts()


def _build():
    nc = bacc.Bacc(debug=False, num_devices=NCORES)

    seq_sl = nc.dram_tensor("seq_sl", [B, C, LC], F32, kind="ExternalInput")
    expv = nc.dram_tensor("expv", [B, G], F32, kind="ExternalInput")
    smalls = nc.dram_tensor("smalls", [128, SM_NCOL], F32, kind="ExternalInput")
    rowsv = nc.dram_tensor("rowsv", [1, RW_NCOL], F32, kind="ExternalInput")
    w1a = nc.dram_tensor("w1a", [C, 4 * C], FP16, kind="ExternalInput")   # W1.T
    w2a = nc.dram_tensor("w2a", [4 * C, C], FP16, kind="ExternalInput")   # W2.T
    woa = nc.dram_tensor("woa", [C, C], FP16, kind="ExternalInput")       # Wo'.T
    out_sl = nc.dram_tensor("out_sl", [B, C, LC], F32, kind="ExternalOutput")
    dbg = {}
    if DBG:
        dbg["tt_sb"] = nc.dram_tensor("d_ttsb", [8, T], F32, kind="ExternalOutput")
        dbg["tt"] = nc.dram_tensor("d_tt", [128, FP], F32, kind="ExternalOutput")
        dbg["lnz"] = nc.dram_tensor("d_lnz", [MN, B], F32, kind="ExternalOutput")
        dbg["cbb"] = nc.dram_tensor("d_cbb", [128, KD], F32, kind="ExternalOutput")
        dbg["wp"] = nc.dram_tensor("d_wp", [128, FP], F32, kind="ExternalOutput")
        dbg["wH"] = nc.dram_tensor("d_wH", [H, T], F32, kind="ExternalOutput")
        dbg["y"] = nc.dram_tensor("d_y", [KC, 128, T], FP16, kind="ExternalOutput")
        dbg["x"] = nc.dram_tensor("d_x", [KC, 128, T], FP16, kind="ExternalOutput")
        dbg["h0"] = nc.dram_tensor("d_h0", [128, T], FP16, kind="ExternalOutput")
        dbg["y2"] = nc.dram_tensor("d_y2", [KC, 128, T], FP16, kind="ExternalOutput")
        dbg["rstd1"] = nc.dram_tensor("d_rstd1", [1, T], F32, kind="ExternalOutput")

    with tile.TileContext(nc) as tc, ExitStack() as ctx:
        p_w = ctx.enter_context(tc.tile_pool(name="w", bufs=1))
        p_act = ctx.enter_context(tc.tile_pool(name="act", bufs=1))
        p_sm = ctx.enter_context(tc.tile_pool(name="sm", bufs=1))
        ps_mm = ctx.enter_context(tc.tile_pool(name="psmm", bufs=2, space="PSUM"))
        ps_xa = ctx.enter_context(tc.tile_pool(name="psxa", bufs=2, space="PSUM"))
        ps_st = ctx.enter_context(tc.tile_pool(name="psst", bufs=1, space="PSUM"))
        ps_pa = ctx.enter_context(tc.tile_pool(name="pspa", bufs=1, space="PSUM"))
        ps_ck = ctx.enter_context(tc.tile_pool(name="psck", bufs=1, space="PSUM"))

        # ---- tiny on-chip constants (no DMA) -----------------------------
        wtile_f = p_sm.tile([128, T], F32, tag="warmf")
        nc.vector.memset(wtile_f[:], 0.0)
        wtile = p_sm.tile([128, T], F32R, tag="warm")
        nc.vector.tensor_copy(wtile[:], wtile_f[:])
        onesk = p_sm.tile([128, 1], FP16, tag="onesk")
        nc.vector.memset(onesk[:], 1.0 / C)
        eps_col = p_sm.tile([1, 1], F32, tag="epsc")
        nc.vector.memset(eps_col[:], EPS)

        # ---- DMA loads: 5 independent queue rows -------------------------
        evb = p_sm.tile([1, B * G], F32, tag="evb")
        nc.scalar.dma_start(evb[:], expv[:])
        sm = p_sm.tile([128, SM_NCOL], F32, tag="sm")
        nc.scalar.dma_start(sm[:], smalls[:])
        eb = p_act.tile([MN, B * G], F32, tag="eb")
        for b in range(B):
            nc.gpsimd.partition_broadcast(eb[0:MN, b * G:(b + 1) * G],
                                          evb[0:1, b * G:(b + 1) * G])

        rows = p_sm.tile([1, RW_NCOL], F32, tag="rows")
        nc.scalar.dma_start(rows[:], rowsv[:])
        xs = p_w.tile([128, KC, B, LC], F32R, tag="xs")
        for b in range(B):
            nc.sync.dma_start(xs[:, :, b, :],
                              seq_sl[b].rearrange("(kt p) l -> p kt l", p=128).bitcast(F32R))
        w1s = p_w.tile([128, KC, 4 * C], FP16, tag="w1")
        nc.sync.dma_start(w1s[:], w1a.rearrange("(kt p) m -> p kt m", p=128))

        w2s = p_w.tile([128, KH, C], FP16, tag="w2")
        nc.gpsimd.dma_start(w2s[:], w2a.rearrange("(kh p) m -> p kh m", p=128))
        wos = p_w.tile([128, KC, C], FP16, tag="wo")
        nc.gpsimd.dma_start(wos[:], woa.rearrange("(kt p) m -> p kt m", p=128))

        # ---- rounded f32r views of small matmul operands -----------------
        m4r = p_sm.tile([128, KC * 8], F32R, tag="m4r")
        nc.vector.tensor_copy(m4r[:], sm[:, SM_M:SM_M + KC * 8])
        rowsr = p_sm.tile([1, RW_NCOL], F32R, tag="rowsr")
        nc.vector.tensor_copy(rowsr[:], rows[:])
        uvhr = p_sm.tile([H, C], F32R, tag="uvhr")
        nc.vector.tensor_copy(uvhr[:], sm[0:H, SM_UVH:SM_UVH + C])

        # ---- PE warm-up while DMAs land ----------------------------------
        for i in range(NWARM):
            pw = ps_pa.tile([8, T], F32, tag="pa", name=f"warm{i}")
            nc.tensor.matmul(pw[:], wtile[:, 0:8], wtile[:], start=True, stop=True)

        # ---- a = x_seq @ M' + a0'  (pre-scaled to t units) ---------------
        pa = ps_ck.tile([8, T], F32, tag="ck", name="pa")
        for kt in range(KC):
            nc.tensor.matmul(pa[:], m4r[:, kt * 8:(kt + 1) * 8],
                             xs[:, kt, :, :],
                             start=(kt == 0), stop=False)
        nc.tensor.matmul(pa[:], rowsr[0:1, RW_A0R:RW_A0R + 8],
                         rowsr[0:1, RW_ONE:RW_ONE + T], start=False, stop=True)
        tt_sb = p_sm.tile([8, T], F32, tag="tts")
        nc.scalar.copy(tt_sb[:], pa[:])

        # ---- logZ sampling at 32 nodes, both batches ---------------------
        z2 = p_sm.tile([MN, B], F32, tag="z2")
        for b in range(B):
            pn = p_act.tile([MN, G], F32, tag="pn", bufs=2, name=f"pn{b}")
            nc.scalar.activation(pn[:], eb[:, b * G:(b + 1) * G], AF.Exp,
                                 scale=sm[0:MN, SM_XN:SM_XN + 1],
                                 accum_out=z2[:, b:b + 1])
        lnz = p_sm.tile([MN, B], F32, tag="lnz")
        nc.scalar.activation(lnz[:], z2[:], AF.Ln)
        sqpre = p_sm.tile([1, 1], F32, tag="sqpre")
        nc.scalar.activation(sqpre[:], eps_col[:], AF.Sqrt, bias=eps_col[:])

        if DBG:
            nc.gpsimd.dma_start(dbg["tt_sb"][:], tt_sb[:])
            nc.gpsimd.dma_start(dbg["lnz"][:], lnz[:])
        # repack to [128, 32], p = b*64 + h*8 + lhi, free = llo (l=lhi*32+llo)
        # (scalar-queue order: after the exp/lnz chain so it doesn't stall it)
        tt = p_sm.tile([128, FP], F32, tag="tt")
        for b in range(B):
            nc.scalar.dma_start(tt[b * 64:(b + 1) * 64, :],
                                tt_sb[:, b * LC:(b + 1) * LC])
        nc.vector.tensor_scalar_max(tt[:], tt[:], -1.0)
        nc.vector.tensor_scalar_min(tt[:], tt[:], 1.0)

        # coeffs: ck2 = dct1.T @ lnz [KD, B]; transpose; broadcast to [128, KD]
        ck2_ps = ps_ck.tile([KD, B], F32, tag="ck")
        nc.tensor.matmul(ck2_ps[:], sm[0:MN, SM_DCT:SM_DCT + KD], lnz[:],
                         start=True, stop=True)
        ck2_sb = p_sm.tile([KD, B], F32, tag="ck2s")
        nc.scalar.copy(ck2_sb[:], ck2_ps[:])
        ckT_ps = ps_ck.tile([B, KD], F32, tag="ck", name="ckT_ps")
        nc.tensor.transpose(ckT_ps[:], ck2_sb[:], sm[0:KD, SM_ID:SM_ID + KD])
        ckT_sb = p_sm.tile([B, KD], F32, tag="ckTs")
        nc.scalar.copy(ckT_sb[:], ckT_ps[:])
        cbb_ps = ps_ck.tile([128, KD], F32, tag="ck", name="cbb_ps")
        nc.tensor.matmul(cbb_ps[:], sm[0:B, SM_SEL:SM_SEL + 128], ckT_sb[:],
                         start=True, stop=True)
        cbb = p_sm.tile([128, KD], F32, tag="cbbs")
        nc.scalar.copy(cbb[:], cbb_ps[:])

        if DBG:
            nc.gpsimd.dma_start(dbg["tt"][:], tt[:])
            nc.gpsimd.dma_start(dbg["cbb"][:], cbb[:])
        # ---- Chebyshev T_k recurrence (vector) ---------------------------
        tt2 = p_sm.tile([128, FP], F32, tag="tt2")
        nc.vector.tensor_add(tt2[:], tt[:], tt[:])
        t_tiles = [None, tt]
        for k in range(2, KD):
            tk = p_sm.tile([128, FP], F32, tag=f"t{k}", name=f"t{k}")
            nc.vector.tensor_mul(tk[:], tt2[:], t_tiles[k - 1][:])
            if k == 2:
                nc.vector.tensor_scalar_sub(tk[:], tk[:], 1.0)   # T0 = 1
            else:
                nc.vector.tensor_sub(tk[:], tk[:], t_tiles[k - 2][:])
            t_tiles.append(tk)
            if k in (5, 8, 11, 14):      # PE keep-warm trickle
                tkr = p_sm.tile([128, 8], F32R, tag="tkr", name=f"tkr{k}")
                nc.gpsimd.tensor_copy(tkr[:], tk[:, 0:8])
                pw = ps_pa.tile([8, T], F32, tag="pa", name=f"trk{k}")
                nc.tensor.matmul(pw[:], tkr[:], wtile[:], start=True, stop=True)
        # t0 term is a constant: handled in the k=1 seed below.

        # ---- contraction sum_k c_k T_k (vector) --------------------------
        accA = p_sm.tile([128, FP], F32, tag="accA")
        accB = p_sm.tile([128, FP], F32, tag="accB")
        nc.vector.tensor_scalar(accA[:], tt[:], cbb[:, 1:2], cbb[:, 0:1],
                                op0=OP.mult, op1=OP.add)
        wp_t = p_sm.tile([128, FP], F32, tag="wp", name="wp")
        cur, nxt = accA, accB
        for k in range(2, KD):
            dst = wp_t if k == KD - 1 else nxt
            nc.vector.scalar_tensor_tensor(
                out=dst[:], in0=t_tiles[k][:], scalar=cbb[:, k:k + 1],
                in1=cur[:], op0=OP.mult, op1=OP.add)
            cur, nxt = dst, cur
        w_pack = cur

        if DBG:
            nc.gpsimd.dma_start(dbg["wp"][:], w_pack[:])
        def trickle(dep, nm):
            tkr = p_sm.tile([128, 8], F32R, tag="tkr", name=f"tkr{nm}")
            nc.gpsimd.tensor_copy(tkr[:], dep[:, 0:8])
            pw = ps_pa.tile([8, T], F32, tag="pa", name=f"trw{nm}")
            nc.tensor.matmul(pw[:], tkr[:], wtile[:], start=True, stop=True)

        def ln_stats_tile(st2, y_tile, kt, ph):
            """Mean contribution inline; squares on gpsimd for a deferred pass."""
            stA, stB, sqs = st2
            nc.tensor.matmul(stA[:], onesk[:], y_tile[:],
                             start=(kt == 0), stop=(kt == KC - 1))
            sq = p_act.tile([128, T], FP16, tag="sq", bufs=4, name=f"sq{ph}{kt}")
            nc.scalar.activation(sq[:], y_tile[:], AF.Square)
            sqs.append(sq)

        def ln_stats_close(st2):
            stA, stB, sqs = st2
            for kt, sq in enumerate(sqs):
                nc.tensor.matmul(stB[:], onesk[:], sq[:],
                                 start=(kt == 0), stop=(kt == KC - 1))

        def ln_stats_open(ph):
            stA = ps_st.tile([1, T], F32, tag="stA", name=f"stA{ph}")
            stB = ps_st.tile([1, T], F32, tag="stB", name=f"stB{ph}")
            return stA, stB, []

        # ---- unpack w to [H, T] and apply: y = w*u_v + c_v + x_seq -------
        wH = p_sm.tile([H, T], F32R, tag="wH")
        for b in range(B):
            nc.scalar.dma_start(wH[:, b * LC:(b + 1) * LC],
                                w_pack[b * 64:(b + 1) * 64, :].bitcast(F32R))
        y_t = []
        st1 = ln_stats_open("a")
        for kt in range(KC):
            xa = ps_xa.tile([128, T], F32, tag="xa", name=f"xa{kt}")
            nc.tensor.matmul(xa[:], uvhr[:, kt * 128:(kt + 1) * 128],
                             wH[:], start=True, stop=True)
            yk = p_act.tile([128, T], FP16, tag="y", bufs=4, name=f"y{kt}")
            eng = nc.vector
            eng.scalar_tensor_tensor(
                out=yk[:], in0=xa[:], scalar=sm[:, SM_CV + kt:SM_CV + kt + 1],
                in1=xs[:, kt, :, :].bitcast(F32), op0=OP.add, op1=OP.add)
            y_t.append(yk)
            ln_stats_tile(st1, yk, kt, "a")
            if kt in (1, 3):
                trickle(yk, f"y{kt}")

        if DBG:
            nc.gpsimd.dma_start(dbg["wH"][:], wH[:].bitcast(F32))
            for kt in range(KC):
                nc.gpsimd.dma_start(dbg["y"][kt], y_t[kt][:])

        def ln_rows(st2, ph, want_mu=False, want_q=False):
            """mean/meansq -> (mu, std, rstd, q=mu*rstd) rows [1, T]."""
            stA, stB = st2[0], st2[1]
            musq = p_sm.tile([1, T], F32, tag="lnr", bufs=6, name=f"musq{ph}")
            nc.scalar.activation(musq[:], stA[:], AF.Square)
            var = p_sm.tile([1, T], F32, tag="lnr", bufs=6, name=f"var{ph}")
            nc.vector.tensor_sub(var[:], stB[:], musq[:])
            std = p_sm.tile([1, T], F32R, tag="lnr", bufs=6, name=f"std{ph}")
            nc.scalar.activation(std[:], var[:], AF.Sqrt, bias=eps_col[:])
            pwln = ps_pa.tile([8, T], F32, tag="pa", name=f"pwln{ph}")
            nc.tensor.matmul(pwln[:], rowsr[0:1, RW_ONE:RW_ONE + 8], std[:],
                             start=True, stop=True)
            rstd_f = p_sm.tile([1, T], F32, tag="rstdf", bufs=2, name=f"rstdf{ph}")
            nc.vector.reciprocal_approx_fast(rstd_f[:], std[:].bitcast(F32))
            rstd = p_sm.tile([1, T], F32R, tag="rstd", bufs=2, name=f"rstd{ph}")
            nc.vector.tensor_copy(rstd[:], rstd_f[:])
            mu = q = None
            if want_mu:
                mu = p_sm.tile([1, T], F32R, tag="mu", bufs=2, name=f"mu{ph}")
                nc.vector.tensor_copy(mu[:], stA[:])
            if want_q:
                q = p_sm.tile([1, T], F32R, tag="q", bufs=2, name=f"q{ph}")
                nc.vector.tensor_mul(q[:], stA[:], rstd_f[:])
            return mu, std, rstd, q

        # ---- LN1 apply -> x ----------------------------------------------
        ln_stats_close(st1)
        _, _, rstd1, q1 = ln_rows(st1, "a", want_q=True)
        x_t = []
        for kt in range(KC):
            sl = slice(RW_G1 + kt * 128, RW_G1 + (kt + 1) * 128)
            pA = ps_mm.tile([128, T], F32, tag="mm", name=f"pA{kt}")
            nc.tensor.matmul(pA[:], rowsr[0:1, sl], rstd1[:],
                             start=True, stop=True)
            pB = ps_mm.tile([128, T], F32, tag="mm", name=f"pB{kt}")
            nc.tensor.matmul(pB[:], rowsr[0:1, sl], q1[:],
                             start=True, stop=True)
            eng = nc.vector
            tx = p_act.tile([128, T], F32, tag="tx", bufs=2, name=f"tx{kt}")
            eng.tensor_mul(tx[:], y_t[kt][:], pA[:])
            xo = p_act.tile([128, T], FP16, tag="x", bufs=4, name=f"x{kt}")
            eng.scalar_tensor_tensor(
                out=xo[:], in0=tx[:], scalar=sm[:, SM_BE1 + kt:SM_BE1 + kt + 1],
                in1=pB[:], op0=OP.add, op1=OP.subtract)
            x_t.append(xo)

        if DBG:
            nc.gpsimd.dma_start(dbg["rstd1"][:], rstd1[:].bitcast(F32))
            for kt in range(KC):
                nc.gpsimd.dma_start(dbg["x"][kt], x_t[kt][:])
        # ---- FFN1: h = relu(W1 @ x + b1) ---------------------------------
        h_t = []
        for mt in range(KH):
            pf = ps_mm.tile([128, T], F32, tag="mm", name=f"pf1{mt}")
            for kt in range(KC):
                nc.tensor.matmul(pf[:], w1s[:, kt, mt * 128:(mt + 1) * 128],
                                 x_t[kt][:], start=(kt == 0), stop=(kt == KC - 1))
            hm = p_act.tile([128, T], FP16, tag="h", bufs=KH, name=f"h{mt}")
            nc.scalar.activation(hm[:], pf[:], AF.Relu,
                                 bias=sm[:, SM_B1 + mt:SM_B1 + mt + 1])
            h_t.append(hm)

        if DBG:
            nc.gpsimd.dma_start(dbg["h0"][:], h_t[0][:])
        # ---- FFN2 + residual -> y2 ---------------------------------------
        y2_t = []
        st2 = ln_stats_open("b")
        for mt in range(KC):
            pf = ps_mm.tile([128, T], F32, tag="mm", name=f"pf2{mt}")
            for kh in range(KH):
                nc.tensor.matmul(pf[:], w2s[:, kh, mt * 128:(mt + 1) * 128],
                                 h_t[kh][:], start=(kh == 0), stop=(kh == KH - 1))
            y2 = p_act.tile([128, T], FP16, tag="y2", bufs=4, name=f"y2{mt}")
            eng = nc.vector
            eng.scalar_tensor_tensor(
                out=y2[:], in0=x_t[mt][:], scalar=sm[:, SM_B2 + mt:SM_B2 + mt + 1],
                in1=pf[:], op0=OP.add, op1=OP.add)
            y2_t.append(y2)
            ln_stats_tile(st2, y2, mt, "b")

        if DBG:
            for mt in range(KC):
                nc.gpsimd.dma_start(dbg["y2"][mt], y2_t[mt][:])
        # ---- LN2 folded into output projection ---------------------------
        ln_stats_close(st2)
        mu2, std2, rstd2, _ = ln_rows(st2, "b", want_mu=True)
        rb_ps = ps_xa.tile([128, T], F32, tag="xa", name="rb")
        nc.tensor.matmul(rb_ps[:], rowsr[0:1, RW_ONE:RW_ONE + 128],
                         rstd2[:], start=True, stop=True)
        rb_sb = p_sm.tile([128, T], F32, tag="rbs")
        nc.vector.tensor_copy(rb_sb[:], rb_ps[:])
        for mt in range(KC):
            po = ps_mm.tile([128, T], F32, tag="mm", name=f"po{mt}")
            for kt in range(KC):
                nc.tensor.matmul(po[:], wos[:, kt, mt * 128:(mt + 1) * 128],
                                 y2_t[kt][:], start=(kt == 0), stop=False)
            nc.tensor.matmul(po[:], rowsr[0:1, RW_NS2 + mt * 128:RW_NS2 + (mt + 1) * 128],
                             mu2[:], start=False, stop=False)
            nc.tensor.matmul(po[:], rowsr[0:1, RW_BO + mt * 128:RW_BO + (mt + 1) * 128],
                             std2[:], start=False, stop=True)
            om = p_act.tile([128, T], F32, tag="om", bufs=2, name=f"om{mt}")
            nc.vector.tensor_mul(om[:], po[:], rb_sb[:])
            seng = nc.sync if mt % 2 == 0 else nc.gpsimd
            seng.dma_start(out_sl[:, mt * 128:(mt + 1) * 128, :].rearrange("b c l -> c b l"),
                           om[:])

    nc.compile()
    return nc


def _host_pack(inputs):
    f32 = lambda x: np.asarray(x, dtype=np.float32)
    Wq, Wk, Wv, Wo = (f32(inputs[k]) for k in ("Wq", "Wk", "Wv", "Wo"))
    W1, W2 = f32(inputs["W1"]), f32(inputs["W2"])
    Wg = f32(inputs["Wg"])[:, 0]
    bg, bq, bv, b1, b2, bo = (f32(inputs[k]) for k in ("bg", "bq", "bv", "b1", "b2", "bo"))
    g1, be1, g2, be2 = (f32(inputs[k]) for k in ("g1", "beta1", "g2", "beta2"))

    u_k = Wk @ Wg
    u_v = Wv @ Wg
    c_v = Wv @ bg + bv
    M = np.zeros((C, H), np.float32)
    a0 = np.zeros(H, np.float32)
    for h in range(H):
        ukh = u_k[h * D:(h + 1) * D]
        M[:, h] = Wq[h * D:(h + 1) * D, :].T @ ukh
        a0[h] = bq[h * D:(h + 1) * D] @ ukh
    Mp = M * (SCALE / SCAL)
    a0p = a0 * (SCALE / SCAL)
    uvH = np.zeros((H, C), np.float32)
    for h in range(H):
        uvH[h, h * D:(h + 1) * D] = u_v[h * D:(h + 1) * D]
    Wop = Wo * g2[None, :]
    bop = bo + Wo @ be2
    s2 = Wop.sum(1)

    smalls = np.zeros((128, SM_NCOL), np.float32)
    for kt in range(KC):
        smalls[:, SM_M + kt * 8:SM_M + (kt + 1) * 8] = Mp[kt * 128:(kt + 1) * 128, :]
        smalls[:, SM_CV + kt] = c_v[kt * 128:(kt + 1) * 128]
        smalls[:, SM_B2 + kt] = b2[kt * 128:(kt + 1) * 128]
        smalls[:, SM_BE1 + kt] = be1[kt * 128:(kt + 1) * 128]
    smalls[0:H, SM_UVH:SM_UVH + C] = uvH
    smalls[0:MN, SM_DCT:SM_DCT + KD] = _DCT1
    smalls[0:KD, SM_ID:SM_ID + KD] = np.eye(KD, dtype=np.float32)
    for p in range(128):
        smalls[p // 64, SM_SEL + p] = 1.0
    smalls[0:MN, SM_XN] = _XN
    smalls[0:H, SM_A0] = a0p
    for mt in range(KH):
        smalls[:, SM_B1 + mt] = b1[mt * 128:(mt + 1) * 128]

    rowsv = np.zeros((1, RW_NCOL), np.float32)
    rowsv[0, RW_G1:RW_G1 + C] = g1
    rowsv[0, RW_NS2:RW_NS2 + C] = -s2
    rowsv[0, RW_ONE:RW_ONE + 512] = 1.0
    rowsv[0, RW_BO:RW_BO + C] = bop
    rowsv[0, RW_A0R:RW_A0R + H] = a0p

    f16t = lambda x: np.ascontiguousarray(x.T, dtype=np.float16)
    return {
        "expv": f32(inputs["exp"]),
        "smalls": smalls,
        "rowsv": rowsv,
        "w1a": f16t(W1),
        "w2a": f16t(W2),
        "woa": f16t(Wop),
    }


def kernel(**inputs):
    global _CACHE, LAST_RESULTS
    if _CACHE is None:
        _CACHE = _build()
    nc = _CACHE

    base = _host_pack(inputs)
    seq = np.asarray(inputs["seq"], dtype=np.float32)
    in_maps = []
    for c in range(NCORES):
        m = dict(base)
        m["seq_sl"] = np.ascontiguousarray(seq[:, :, c * LC:(c + 1) * LC])
        in_maps.append(m)

    res = run_bass_kernel_spmd(nc, in_maps, list(range(NCORES)), trace=TRACE,
                               **TRACE_KW)
    LAST_RESULTS = res
    out = np.empty((B, C, L), np.float32)
    for c in range(NCORES):
        out[:, :, c * LC:(c + 1) * LC] = res.results[c]["out_sl"]
    return out


# revision 26
# speedup vs baseline: 1.0346x; 1.0281x over previous
"""Trainium2 Bass kernel for nn_G3DCrossAttention (B=2, C=512, L=2048, G=2048, H=8).

Algebraic structure (exact math): exp_p[g,b,:] = exp[b,g]*Wg[:,0]+bg is rank-1, so
k/v collapse to k = e*u_k + c_k, v = e*u_v + c_v.  The j-constant score shift
cancels in softmax, the attention output collapses per head to
    x_attn = w*u_v + c_v,   w_i = f_b(a_i),  a = x_seq @ M + a0,
with f_b(a) = d/da log Z_b(a),  Z_b(a) = sum_j exp(a*e_bj).  On device, log Z is
sampled at 32 Chebyshev nodes (exp + accum), and a host-precomputed linear map
(fit + analytic series derivative) turns those samples into degree-20 Chebyshev
coefficients of f_b.  f is evaluated at all (i,h) via a T_k recurrence in a
packed [128,32] layout, unpacked to [H,T] by one SBUF->SBUF DMA, and applied as
one outer-product matmul per 128-channel tile.

All weight-only transforms (u_k/u_v/c_v, M, a0, LN2 folded into Wo'=Wo*g2,
bo'=bo+Wo@be2, s2=Wo'@1) are computed on HOST; the device sees three fp16
weight mats (W1.T, W2.T, Wo'.T), the f32 seq slice, exp, and one packed
[128,742]+[1,2176] constant grid.  LN2's normalization is folded into the
output projection: out = rstd2 .* (Wo'@y2 - s2(x)mu2 + bo'(x)std2).

Sharding: data-parallel over L (LC=256 queries/core), full pipeline per core.
"""

from contextlib import ExitStack

import numpy as np

import concourse.bass as bass
import concourse.tile as tile
from concourse import bacc, mybir
from concourse.bass_utils import run_bass_kernel_spmd

F32 = mybir.dt.float32
F32R = mybir.dt.float32r
FP16 = mybir.dt.float16
AF = mybir.ActivationFunctionType
OP = mybir.AluOpType

B, C, L, G, H = 2, 512, 2048, 2048, 8
D = C // H
NCORES = 8
LC = L // NCORES              # 256 queries per core
T = B * LC                    # 512 tokens per core, tau = b*LC + l
KC = C // 128                 # 4
KH = (4 * C) // 128           # 16
FP = 32                       # llo width of the packed a/w layout
NLHI = LC // FP               # 8
SCALE = 1.0 / float(np.sqrt(D))
EPS = 1e-5
SCAL = 5.0                    # Chebyshev half-range in a units (|a|max ~ 4.43)
KD = 16                       # Chebyshev series length for f = (logZ)'
MN = 32                       # logZ sample nodes per batch
NWARM = 7                     # PE warm-up matmuls while DMAs land

# ---- smalls grid column layout (f32 [128, SM_NCOL]) -------------------------
SM_M = 0                      # [128, 32]  M' tiles (kt-major, 8 cols each)
SM_UVH = 32                   # [8, 512]   u_v gathered into head rows
SM_DCT = 544                  # [32, KD]   logZ samples -> f coeffs
SM_ID = 564                   # [KD, KD]   identity for PE transpose
SM_SEL = 584                  # [2, 128]   batch selector for coeff broadcast
SM_XN = 712                   # [32, 1]    a-space Chebyshev nodes
SM_A0 = 713                   # [8, 1]     a0' bias
SM_CV = 714                   # [128, 4]   c_v per kt tile
SM_B1 = 718                   # [128, 16]  b1 per mt tile
SM_B2 = 734                   # [128, 4]
SM_BE1 = 738                  # [128, 4]
SM_NCOL = 742

# ---- rows vector layout (f32 [1, RW_NCOL]) ----------------------------------
RW_G1 = 0                     # g1 [C]
RW_NS2 = 512                  # -s2 [C]
RW_ONE = 1024                 # ones [512]
RW_BO = 1536                  # bo' [C]
RW_A0R = 2048                 # a0' [8]
RW_NCOL = 2056

TRACE = False
TRACE_KW = {}
LAST_RESULTS = None
_CACHE = None
DBG = False


def _host_consts():
    """Input-independent matrices for the smalls grid."""
    m = np.arange(MN)
    theta = np.pi * (2 * m + 1) / (2 * MN)
    xn = (SCAL * np.cos(theta)).astype(np.float32)          # nodes in a units
    F = np.zeros((KD, MN))
    for k in range(KD):
        F[k] = (2.0 / MN) * np.cos(k * theta)
    F[0] *= 0.5
    import numpy.polynomial.chebyshev as Ch
    DER = np.zeros((KD, KD))
    for k in range(KD):
        ck = np.zeros(KD)
        ck[k] = 1
        dd = Ch.chebder(ck)
        DER[:len(dd), k] = dd
    DM = (DER @ F) / SCAL                                   # [KD, MN]
    return xn, DM.T.astype(np.float32)                      # dct1 [MN, KD]


_XN, _DCT1 = _host_consts()


def _build():
    nc = bacc.Bacc(debug=False, num_devices=NCORES)

    seq_sl = nc.dram_tensor("seq_sl", [B, C, LC], F32, kind="ExternalInput")
    expv = nc.dram_tensor("expv", [B, G], F32, kind="ExternalInput")
    smalls = nc.dram_tensor("smalls", [128, SM_NCOL], F32, kind="ExternalInput")
    rowsv = nc.dram_tensor("rowsv", [1, RW_NCOL], F32, kind="ExternalInput")
    w1a = nc.dram_tensor("w1a", [C, 4 * C], FP16, kind="ExternalInput")   # W1.T
    w2a = nc.dram_tensor("w2a", [4 * C, C], FP16, kind="ExternalInput")   # W2.T
    woa = nc.dram_tensor("woa", [C, C], FP16, kind="ExternalInput")       # Wo'.T
    out_sl = nc.dram_tensor("out_sl", [B, C, LC], F32, kind="ExternalOutput")
    dbg = {}
    if DBG:
        dbg["tt_sb"] = nc.dram_tensor("d_ttsb", [8, T], F32, kind="ExternalOutput")
        dbg["tt"] = nc.dram_tensor("d_tt", [128, FP], F32, kind="ExternalOutput")
        dbg["lnz"] = nc.dram_tensor("d_lnz", [MN, B], F32, kind="ExternalOutput")
        dbg["cbb"] = nc.dram_tensor("d_cbb", [128, KD], F32, kind="ExternalOutput")
        dbg["wp"] = nc.dram_tensor("d_wp", [128, FP], F32, kind="ExternalOutput")
        dbg["wH"] = nc.dram_tensor("d_wH", [H, T], F32, kind="ExternalOutput")
        dbg["y"] = nc.dram_tensor("d_y", [KC, 128, T], FP16, kind="ExternalOutput")
        dbg["x"] = nc.dram_tensor("d_x", [KC, 128, T], FP16, kind="ExternalOutput")
        dbg["h0"] = nc.dram_tensor("d_h0", [128, T], FP16, kind="ExternalOutput")
        dbg["y2"] = nc.dram_tensor("d_y2", [KC, 128, T], FP16, kind="ExternalOutput")
        dbg["rstd1"] = nc.dram_tensor("d_rstd1", [1, T], F32, kind="ExternalOutput")

    with tile.TileContext(nc) as tc, ExitStack() as ctx:
        p_w = ctx.enter_context(tc.tile_pool(name="w", bufs=1))
        p_act = ctx.enter_context(tc.tile_pool(name="act", bufs=1))
        p_sm = ctx.enter_context(tc.tile_pool(name="sm", bufs=1))
        ps_mm = ctx.enter_context(tc.tile_pool(name="psmm", bufs=2, space="PSUM"))
        ps_xa = ctx.enter_context(tc.tile_pool(name="psxa", bufs=2, space="PSUM"))
        ps_st = ctx.enter_context(tc.tile_pool(name="psst", bufs=1, space="PSUM"))
        ps_pa = ctx.enter_context(tc.tile_pool(name="pspa", bufs=1, space="PSUM"))
        ps_ck = ctx.enter_context(tc.tile_pool(name="psck", bufs=1, space="PSUM"))

        # ---- tiny on-chip constants (no DMA) -----------------------------
        wtile_f = p_sm.tile([128, T], F32, tag="warmf")
        nc.vector.memset(wtile_f[:], 0.0)
        wtile = p_sm.tile([128, T], F32R, tag="warm")
        nc.vector.tensor_copy(wtile[:], wtile_f[:])
        onesk = p_sm.tile([128, 1], FP16, tag="onesk")
        nc.vector.memset(onesk[:], 1.0 / C)
        eps_col = p_sm.tile([1, 1], F32, tag="epsc")
        nc.vector.memset(eps_col[:], EPS)

        # ---- DMA loads: 5 independent queue rows -------------------------
        sm = p_sm.tile([128, SM_NCOL], F32, tag="sm")
        nc.scalar.dma_start(sm[:], smalls[:])
        eb = p_act.tile([MN, B * G], F32, tag="eb")
        for b in range(B):
            nc.gpsimd.dma_start(eb[0:MN, b * G:(b + 1) * G],
                                expv[b, :][None, :].to_broadcast((MN, G)))

        rows = p_sm.tile([1, RW_NCOL], F32, tag="rows")
        nc.scalar.dma_start(rows[:], rowsv[:])
        xs = p_w.tile([128, KC, B, LC], F32R, tag="xs")
        for b in range(B):
            nc.sync.dma_start(xs[:, :, b, :],
                              seq_sl[b].rearrange("(kt p) l -> p kt l", p=128).bitcast(F32R))
        w1s = p_w.tile([128, KC, 4 * C], FP16, tag="w1")
        nc.sync.dma_start(w1s[:], w1a.rearrange("(kt p) m -> p kt m", p=128))

        w2s = p_w.tile([128, KH, C], FP16, tag="w2")
        nc.gpsimd.dma_start(w2s[:], w2a.rearrange("(kh p) m -> p kh m", p=128))
        wos = p_w.tile([128, KC, C], FP16, tag="wo")
        nc.gpsimd.dma_start(wos[:], woa.rearrange("(kt p) m -> p kt m", p=128))

        # ---- rounded f32r views of small matmul operands -----------------
        m4r = p_sm.tile([128, KC * 8], F32R, tag="m4r")
        nc.vector.tensor_copy(m4r[:], sm[:, SM_M:SM_M + KC * 8])
        rowsr = p_sm.tile([1, RW_NCOL], F32R, tag="rowsr")
        nc.vector.tensor_copy(rowsr[:], rows[:])
        uvhr = p_sm.tile([H, C], F32R, tag="uvhr")
        nc.vector.tensor_copy(uvhr[:], sm[0:H, SM_UVH:SM_UVH + C])

        # ---- PE warm-up while DMAs land ----------------------------------
        for i in range(NWARM):
            pw = ps_pa.tile([8, T], F32, tag="pa", name=f"warm{i}")
            nc.tensor.matmul(pw[:], wtile[:, 0:8], wtile[:], start=True, stop=True)

        # ---- a = x_seq @ M' + a0'  (pre-scaled to t units) ---------------
        pa = ps_ck.tile([8, T], F32, tag="ck", name="pa")
        for kt in range(KC):
            nc.tensor.matmul(pa[:], m4r[:, kt * 8:(kt + 1) * 8],
                             xs[:, kt, :, :],
                             start=(kt == 0), stop=False)
        nc.tensor.matmul(pa[:], rowsr[0:1, RW_A0R:RW_A0R + 8],
                         rowsr[0:1, RW_ONE:RW_ONE + T], start=False, stop=True)
        tt_sb = p_sm.tile([8, T], F32, tag="tts")
        nc.scalar.copy(tt_sb[:], pa[:])

        # ---- logZ sampling at 32 nodes, both batches ---------------------
        z2 = p_sm.tile([MN, B], F32, tag="z2")
        for b in range(B):
            pn = p_act.tile([MN, G], F32, tag="pn", bufs=2, name=f"pn{b}")
            nc.scalar.activation(pn[:], eb[:, b * G:(b + 1) * G], AF.Exp,
                                 scale=sm[0:MN, SM_XN:SM_XN + 1],
                                 accum_out=z2[:, b:b + 1])
        lnz = p_sm.tile([MN, B], F32, tag="lnz")
        nc.scalar.activation(lnz[:], z2[:], AF.Ln)
        sqpre = p_sm.tile([1, 1], F32, tag="sqpre")
        nc.scalar.activation(sqpre[:], eps_col[:], AF.Sqrt, bias=eps_col[:])

        if DBG:
            nc.gpsimd.dma_start(dbg["tt_sb"][:], tt_sb[:])
            nc.gpsimd.dma_start(dbg["lnz"][:], lnz[:])
        # repack to [128, 32], p = b*64 + h*8 + lhi, free = llo (l=lhi*32+llo)
        # (scalar-queue order: after the exp/lnz chain so it doesn't stall it)
        tt = p_sm.tile([128, FP], F32, tag="tt")
        for b in range(B):
            nc.scalar.dma_start(tt[b * 64:(b + 1) * 64, :],
                                tt_sb[:, b * LC:(b + 1) * LC])
        nc.vector.tensor_scalar_max(tt[:], tt[:], -1.0)
        nc.vector.tensor_scalar_min(tt[:], tt[:], 1.0)

        # coeffs: ck2 = dct1.T @ lnz [KD, B]; transpose; broadcast to [128, KD]
        ck2_ps = ps_ck.tile([KD, B], F32, tag="ck")
        nc.tensor.matmul(ck2_ps[:], sm[0:MN, SM_DCT:SM_DCT + KD], lnz[:],
                         start=True, stop=True)
        ck2_sb = p_sm.tile([KD, B], F32, tag="ck2s")
        nc.scalar.copy(ck2_sb[:], ck2_ps[:])
        ckT_ps = ps_ck.tile([B, KD], F32, tag="ck", name="ckT_ps")
        nc.tensor.transpose(ckT_ps[:], ck2_sb[:], sm[0:KD, SM_ID:SM_ID + KD])
        ckT_sb = p_sm.tile([B, KD], F32, tag="ckTs")
        nc.scalar.copy(ckT_sb[:], ckT_ps[:])
        cbb_ps = ps_ck.tile([128, KD], F32, tag="ck", name="cbb_ps")
        nc.tensor.matmul(cbb_ps[:], sm[0:B, SM_SEL:SM_SEL + 128], ckT_sb[:],
                         start=True, stop=True)
        cbb = p_sm.tile([128, KD], F32, tag="cbbs")
        nc.scalar.copy(cbb[:], cbb_ps[:])

        if DBG:
            nc.gpsimd.dma_start(dbg["tt"][:], tt[:])
            nc.gpsimd.dma_start(dbg["cbb"][:], cbb[:])
        # ---- Chebyshev T_k recurrence (vector) ---------------------------
        tt2 = p_sm.tile([128, FP], F32, tag="tt2")
        nc.vector.tensor_add(tt2[:], tt[:], tt[:])
        t_tiles = [None, tt]
        for k in range(2, KD):
            tk = p_sm.tile([128, FP], F32, tag=f"t{k}", name=f"t{k}")
            nc.vector.tensor_mul(tk[:], tt2[:], t_tiles[k - 1][:])
            if k == 2:
                nc.vector.tensor_scalar_sub(tk[:], tk[:], 1.0)   # T0 = 1
            else:
                nc.vector.tensor_sub(tk[:], tk[:], t_tiles[k - 2][:])
            t_tiles.append(tk)
            if k in (5, 8, 11, 14):      # PE keep-warm trickle
                tkr = p_sm.tile([128, 8], F32R, tag="tkr", name=f"tkr{k}")
                nc.gpsimd.tensor_copy(tkr[:], tk[:, 0:8])
                pw = ps_pa.tile([8, T], F32, tag="pa", name=f"trk{k}")
                nc.tensor.matmul(pw[:], tkr[:], wtile[:], start=True, stop=True)
        # t0 term is a constant: handled in the k=1 seed below.

        # ---- contraction sum_k c_k T_k (vector) --------------------------
        accA = p_sm.tile([128, FP], F32, tag="accA")
        accB = p_sm.tile([128, FP], F32, tag="accB")
        nc.vector.tensor_scalar(accA[:], tt[:], cbb[:, 1:2], cbb[:, 0:1],
                                op0=OP.mult, op1=OP.add)
        wp_t = p_sm.tile([128, FP], F32, tag="wp", name="wp")
        cur, nxt = accA, accB
        for k in range(2, KD):
            dst = wp_t if k == KD - 1 else nxt
            nc.vector.scalar_tensor_tensor(
                out=dst[:], in0=t_tiles[k][:], scalar=cbb[:, k:k + 1],
                in1=cur[:], op0=OP.mult, op1=OP.add)
            cur, nxt = dst, cur
        w_pack = cur

        if DBG:
            nc.gpsimd.dma_start(dbg["wp"][:], w_pack[:])
        def trickle(dep, nm):
            tkr = p_sm.tile([128, 8], F32R, tag="tkr", name=f"tkr{nm}")
            nc.gpsimd.tensor_copy(tkr[:], dep[:, 0:8])
            pw = ps_pa.tile([8, T], F32, tag="pa", name=f"trw{nm}")
            nc.tensor.matmul(pw[:], tkr[:], wtile[:], start=True, stop=True)

        def ln_stats_tile(st2, y_tile, kt, ph):
            """Mean contribution inline; squares on gpsimd for a deferred pass."""
            stA, stB, sqs = st2
            nc.tensor.matmul(stA[:], onesk[:], y_tile[:],
                             start=(kt == 0), stop=(kt == KC - 1))
            sq = p_act.tile([128, T], FP16, tag="sq", bufs=4, name=f"sq{ph}{kt}")
            nc.scalar.activation(sq[:], y_tile[:], AF.Square)
            sqs.append(sq)

        def ln_stats_close(st2):
            stA, stB, sqs = st2
            for kt, sq in enumerate(sqs):
                nc.tensor.matmul(stB[:], onesk[:], sq[:],
                                 start=(kt == 0), stop=(kt == KC - 1))

        def ln_stats_open(ph):
            stA = ps_st.tile([1, T], F32, tag="stA", name=f"stA{ph}")
            stB = ps_st.tile([1, T], F32, tag="stB", name=f"stB{ph}")
            return stA, stB, []

        # ---- unpack w to [H, T] and apply: y = w*u_v + c_v + x_seq -------
        wH = p_sm.tile([H, T], F32R, tag="wH")
        for b in range(B):
            nc.scalar.dma_start(wH[:, b * LC:(b + 1) * LC],
                                w_pack[b * 64:(b + 1) * 64, :].bitcast(F32R))
        y_t = []
        st1 = ln_stats_open("a")
        for kt in range(KC):
            xa = ps_xa.tile([128, T], F32, tag="xa", name=f"xa{kt}")
            nc.tensor.matmul(xa[:], uvhr[:, kt * 128:(kt + 1) * 128],
                             wH[:], start=True, stop=True)
            yk = p_act.tile([128, T], FP16, tag="y", bufs=4, name=f"y{kt}")
            eng = nc.vector
            eng.scalar_tensor_tensor(
                out=yk[:], in0=xa[:], scalar=sm[:, SM_CV + kt:SM_CV + kt + 1],
                in1=xs[:, kt, :, :].bitcast(F32), op0=OP.add, op1=OP.add)
            y_t.append(yk)
            ln_stats_tile(st1, yk, kt, "a")
            if kt in (1, 3):
                trickle(yk, f"y{kt}")

        if DBG:
            nc.gpsimd.dma_start(dbg["wH"][:], wH[:].bitcast(F32))
            for kt in range(KC):
                nc.gpsimd.dma_start(dbg["y"][kt], y_t[kt][:])

        def ln_rows(st2, ph, want_mu=False, want_q=False):
            """mean/meansq -> (mu, std, rstd, q=mu*rstd) rows [1, T]."""
            stA, stB = st2[0], st2[1]
            musq = p_sm.tile([1, T], F32, tag="lnr", bufs=6, name=f"musq{ph}")
            nc.scalar.activation(musq[:], stA[:], AF.Square)
            var = p_sm.tile([1, T], F32, tag="lnr", bufs=6, name=f"var{ph}")
            nc.vector.tensor_sub(var[:], stB[:], musq[:])
            std = p_sm.tile([1, T], F32R, tag="lnr", bufs=6, name=f"std{ph}")
            nc.scalar.activation(std[:], var[:], AF.Sqrt, bias=eps_col[:])
            pwln = ps_pa.tile([8, T], F32, tag="pa", name=f"pwln{ph}")
            nc.tensor.matmul(pwln[:], rowsr[0:1, RW_ONE:RW_ONE + 8], std[:],
                             start=True, stop=True)
            rstd_f = p_sm.tile([1, T], F32, tag="rstdf", bufs=2, name=f"rstdf{ph}")
            nc.vector.reciprocal_approx_fast(rstd_f[:], std[:].bitcast(F32))
            rstd = p_sm.tile([1, T], F32R, tag="rstd", bufs=2, name=f"rstd{ph}")
            nc.vector.tensor_copy(rstd[:], rstd_f[:])
            mu = q = None
            if want_mu:
                mu = p_sm.tile([1, T], F32R, tag="mu", bufs=2, name=f"mu{ph}")
                nc.vector.tensor_copy(mu[:], stA[:])
            if want_q:
                q = p_sm.tile([1, T], F32R, tag="q", bufs=2, name=f"q{ph}")
                nc.vector.tensor_mul(q[:], stA[:], rstd_f[:])
            return mu, std, rstd, q

        # ---- LN1 apply -> x ----------------------------------------------
        ln_stats_close(st1)
        _, _, rstd1, q1 = ln_rows(st1, "a", want_q=True)
        x_t = []
        for kt in range(KC):
            sl = slice(RW_G1 + kt * 128, RW_G1 + (kt + 1) * 128)
            pA = ps_mm.tile([128, T], F32, tag="mm", name=f"pA{kt}")
            nc.tensor.matmul(pA[:], rowsr[0:1, sl], rstd1[:],
                             start=True, stop=True)
            pB = ps_mm.tile([128, T], F32, tag="mm", name=f"pB{kt}")
            nc.tensor.matmul(pB[:], rowsr[0:1, sl], q1[:],
                             start=True, stop=True)
            eng = nc.vector
            tx = p_act.tile([128, T], F32, tag="tx", bufs=2, name=f"tx{kt}")
            eng.tensor_mul(tx[:], y_t[kt][:], pA[:])
            xo = p_act.tile([128, T], FP16, tag="x", bufs=4, name=f"x{kt}")
            eng.scalar_tensor_tensor(
                out=xo[:], in0=tx[:], scalar=sm[:, SM_BE1 + kt:SM_BE1 + kt + 1],
                in1=pB[:], op0=OP.add, op1=OP.subtract)
            x_t.append(xo)

        if DBG:
            nc.gpsimd.dma_start(dbg["rstd1"][:], rstd1[:].bitcast(F32))
            for kt in range(KC):
                nc.gpsimd.dma_start(dbg["x"][kt], x_t[kt][:])
        # ---- FFN1: h = relu(W1 @ x + b1) ---------------------------------
        h_t = []
        for mt in range(KH):
            pf = ps_mm.tile([128, T], F32, tag="mm", name=f"pf1{mt}")
            for kt in range(KC):
                nc.tensor.matmul(pf[:], w1s[:, kt, mt * 128:(mt + 1) * 128],
                                 x_t[kt][:], start=(kt == 0), stop=(kt == KC - 1))
            hm = p_act.tile([128, T], FP16, tag="h", bufs=KH, name=f"h{mt}")
            nc.scalar.activation(hm[:], pf[:], AF.Relu,
                                 bias=sm[:, SM_B1 + mt:SM_B1 + mt + 1])
            h_t.append(hm)

        if DBG:
            nc.gpsimd.dma_start(dbg["h0"][:], h_t[0][:])
        # ---- FFN2 + residual -> y2 ---------------------------------------
        y2_t = []
        st2 = ln_stats_open("b")
        for mt in range(KC):
            pf = ps_mm.tile([128, T], F32, tag="mm", name=f"pf2{mt}")
            for kh in range(KH):
                nc.tensor.matmul(pf[:], w2s[:, kh, mt * 128:(mt + 1) * 128],
                                 h_t[kh][:], start=(kh == 0), stop=(kh == KH - 1))
            y2 = p_act.tile([128, T], FP16, tag="y2", bufs=4, name=f"y2{mt}")
            eng = nc.vector
            eng.scalar_tensor_tensor(
                out=y2[:], in0=x_t[mt][:], scalar=sm[:, SM_B2 + mt:SM_B2 + mt + 1],
                in1=pf[:], op0=OP.add, op1=OP.add)
            y2_t.append(y2)
            ln_stats_tile(st2, y2, mt, "b")

        if DBG:
            for mt in range(KC):
                nc.gpsimd.dma_start(dbg["y2"][mt], y2_t[mt][:])
        # ---- LN2 folded into output projection ---------------------------
        ln_stats_close(st2)
        mu2, std2, rstd2, _ = ln_rows(st2, "b", want_mu=True)
        rb_ps = ps_xa.tile([128, T], F32, tag="xa", name="rb")
        nc.tensor.matmul(rb_ps[:], rowsr[0:1, RW_ONE:RW_ONE + 128],
                         rstd2[:], start=True, stop=True)
        rb_sb = p_sm.tile([128, T], F32, tag="rbs")
        nc.vector.tensor_copy(rb_sb[:], rb_ps[:])
        for mt in range(KC):
            po = ps_mm.tile([128, T], F32, tag="mm", name=f"po{mt}")
            for kt in range(KC):
                nc.tensor.matmul(po[:], wos[:, kt, mt * 128:(mt + 1) * 128],
                                 y2_t[kt][:], start=(kt == 0), stop=False)
            nc.tensor.matmul(po[:], rowsr[0:1, RW_NS2 + mt * 128:RW_NS2 + (mt + 1) * 128],
                             mu2[:], start=False, stop=False)
            nc.tensor.matmul(po[:], rowsr[0:1, RW_BO + mt * 128:RW_BO + (mt + 1) * 128],
                             std2[:], start=False, stop=True)
            om = p_act.tile([128, T], F32, tag="om", bufs=2, name=f"om{mt}")
            nc.vector.tensor_mul(om[:], po[:], rb_sb[:])
            seng = nc.sync if mt % 2 == 0 else nc.gpsimd
            seng.dma_start(out_sl[:, mt * 128:(mt + 1) * 128, :].rearrange("b c l -> c b l"),
                           om[:])

    nc.compile()
    return nc


def _host_pack(inputs):
    f32 = lambda x: np.asarray(x, dtype=np.float32)
    Wq, Wk, Wv, Wo = (f32(inputs[k]) for k in ("Wq", "Wk", "Wv", "Wo"))
    W1, W2 = f32(inputs["W1"]), f32(inputs["W2"])
    Wg = f32(inputs["Wg"])[:, 0]
    bg, bq, bv, b1, b2, bo = (f32(inputs[k]) for k in ("bg", "bq", "bv", "b1", "b2", "bo"))
    g1, be1, g2, be2 = (f32(inputs[k]) for k in ("g1", "beta1", "g2", "beta2"))

    u_k = Wk @ Wg
    u_v = Wv @ Wg
    c_v = Wv @ bg + bv
    M = np.zeros((C, H), np.float32)
    a0 = np.zeros(H, np.float32)
    for h in range(H):
        ukh = u_k[h * D:(h + 1) * D]
        M[:, h] = Wq[h * D:(h + 1) * D, :].T @ ukh
        a0[h] = bq[h * D:(h + 1) * D] @ ukh
    Mp = M * (SCALE / SCAL)
    a0p = a0 * (SCALE / SCAL)
    uvH = np.zeros((H, C), np.float32)
    for h in range(H):
        uvH[h, h * D:(h + 1) * D] = u_v[h * D:(h + 1) * D]
    Wop = Wo * g2[None, :]
    bop = bo + Wo @ be2
    s2 = Wop.sum(1)

    smalls = np.zeros((128, SM_NCOL), np.float32)
    for kt in range(KC):
        smalls[:, SM_M + kt * 8:SM_M + (kt + 1) * 8] = Mp[kt * 128:(kt + 1) * 128, :]
        smalls[:, SM_CV + kt] = c_v[kt * 128:(kt + 1) * 128]
        smalls[:, SM_B2 + kt] = b2[kt * 128:(kt + 1) * 128]
        smalls[:, SM_BE1 + kt] = be1[kt * 128:(kt + 1) * 128]
    smalls[0:H, SM_UVH:SM_UVH + C] = uvH
    smalls[0:MN, SM_DCT:SM_DCT + KD] = _DCT1
    smalls[0:KD, SM_ID:SM_ID + KD] = np.eye(KD, dtype=np.float32)
    for p in range(128):
        smalls[p // 64, SM_SEL + p] = 1.0
    smalls[0:MN, SM_XN] = _XN
    smalls[0:H, SM_A0] = a0p
    for mt in range(KH):
        smalls[:, SM_B1 + mt] = b1[mt * 128:(mt + 1) * 128]

    rowsv = np.zeros((1, RW_NCOL), np.float32)
    rowsv[0, RW_G1:RW_G1 + C] = g1
    rowsv[0, RW_NS2:RW_NS2 + C] = -s2
    rowsv[0, RW_ONE:RW_ONE + 512] = 1.0
    rowsv[0, RW_BO:RW_BO + C] = bop
    rowsv[0, RW_A0R:RW_A0R + H] = a0p

    f16t = lambda x: np.ascontiguousarray(x.T, dtype=np.float16)
    return {
        "expv": f32(inputs["exp"]),
        "smalls": smalls,
        "rowsv": rowsv,
        "w1a": f16t(W1),
        "w2a": f16t(W2),
        "woa": f16t(Wop),
    }


def kernel(**inputs):
    global _CACHE, LAST_RESULTS
    if _CACHE is None:
        _CACHE = _build()
    nc = _CACHE

    base = _host_pack(inputs)
    seq = np.asarray(inputs["seq"], dtype=np.float32)
    in_maps = []
    for c in range(NCORES):
        m = dict(base)
        m["seq_sl"] = np.ascontiguousarray(seq[:, :, c * LC:(c + 1) * LC])
        in_maps.append(m)

    res = run_bass_kernel_spmd(nc, in_maps, list(range(NCORES)), trace=TRACE,
                               **TRACE_KW)
    LAST_RESULTS = res
    out = np.empty((B, C, L), np.float32)
    for c in range(NCORES):
        out[:, :, c * LC:(c + 1) * LC] = res.results[c]["out_sl"]
    return out


# revision 27
# speedup vs baseline: 1.0651x; 1.0295x over previous
"""Trainium2 Bass kernel for nn_G3DCrossAttention (B=2, C=512, L=2048, G=2048, H=8).

Algebraic structure (exact math): exp_p[g,b,:] = exp[b,g]*Wg[:,0]+bg is rank-1, so
k/v collapse to k = e*u_k + c_k, v = e*u_v + c_v.  The j-constant score shift
cancels in softmax, the attention output collapses per head to
    x_attn = w*u_v + c_v,   w_i = f_b(a_i),  a = x_seq @ M + a0,
with f_b(a) = d/da log Z_b(a),  Z_b(a) = sum_j exp(a*e_bj).  On device, log Z is
sampled at 32 Chebyshev nodes (exp + accum), and a host-precomputed linear map
(fit + analytic series derivative) turns those samples into degree-20 Chebyshev
coefficients of f_b.  f is evaluated at all (i,h) via a T_k recurrence in a
packed [128,32] layout, unpacked to [H,T] by one SBUF->SBUF DMA, and applied as
one outer-product matmul per 128-channel tile.

All weight-only transforms (u_k/u_v/c_v, M, a0, LN2 folded into Wo'=Wo*g2,
bo'=bo+Wo@be2, s2=Wo'@1) are computed on HOST; the device sees three fp16
weight mats (W1.T, W2.T, Wo'.T), the f32 seq slice, exp, and one packed
[128,742]+[1,2176] constant grid.  LN2's normalization is folded into the
output projection: out = rstd2 .* (Wo'@y2 - s2(x)mu2 + bo'(x)std2).

Sharding: data-parallel over L (LC=256 queries/core), full pipeline per core.
"""

from contextlib import ExitStack

import numpy as np

import concourse.bass as bass
import concourse.tile as tile
from concourse import bacc, mybir
from concourse.bass_utils import run_bass_kernel_spmd

F32 = mybir.dt.float32
F32R = mybir.dt.float32r
FP16 = mybir.dt.float16
AF = mybir.ActivationFunctionType
OP = mybir.AluOpType

B, C, L, G, H = 2, 512, 2048, 2048, 8
D = C // H
NCORES = 8
LC = L // NCORES              # 256 queries per core
T = B * LC                    # 512 tokens per core, tau = b*LC + l
KC = C // 128                 # 4
KH = (4 * C) // 128           # 16
FP = 32                       # llo width of the packed a/w layout
NLHI = LC // FP               # 8
SCALE = 1.0 / float(np.sqrt(D))
EPS = 1e-5
SCAL = 5.0                    # Chebyshev half-range in a units (|a|max ~ 4.43)
KD = 16                       # Chebyshev series length for f = (logZ)'
MN = 32                       # logZ sample nodes per batch
NWARM = 7                     # PE warm-up matmuls while DMAs land

# ---- smalls grid column layout (f32 [128, SM_NCOL]) -------------------------
SM_M = 0                      # [128, 32]  M' tiles (kt-major, 8 cols each)
SM_UVH = 32                   # [8, 512]   u_v gathered into head rows
SM_DCT = 544                  # [32, KD]   logZ samples -> f coeffs
SM_ID = 564                   # [KD, KD]   identity for PE transpose
SM_SEL = 584                  # [2, 128]   batch selector for coeff broadcast
SM_XN = 712                   # [32, 1]    a-space Chebyshev nodes
SM_A0 = 713                   # [8, 1]     a0' bias
SM_CV = 714                   # [128, 4]   c_v per kt tile
SM_B1 = 718                   # [128, 16]  b1 per mt tile
SM_B2 = 734                   # [128, 4]
SM_BE1 = 738                  # [128, 4]
SM_NCOL = 742

# ---- rows vector layout (f32 [1, RW_NCOL]) ----------------------------------
RW_G1 = 0                     # g1 [C]
RW_NS2 = 512                  # -s2 [C]
RW_ONE = 1024                 # ones [512]
RW_BO = 1536                  # bo' [C]
RW_A0R = 2048                 # a0' [8]
RW_NCOL = 2056

TRACE = False
TRACE_KW = {}
LAST_RESULTS = None
_CACHE = None
DBG = False


def _host_consts():
    """Input-independent matrices for the smalls grid."""
    m = np.arange(MN)
    theta = np.pi * (2 * m + 1) / (2 * MN)
    xn = (SCAL * np.cos(theta)).astype(np.float32)          # nodes in a units
    F = np.zeros((KD, MN))
    for k in range(KD):
        F[k] = (2.0 / MN) * np.cos(k * theta)
    F[0] *= 0.5
    import numpy.polynomial.chebyshev as Ch
    DER = np.zeros((KD, KD))
    for k in range(KD):
        ck = np.zeros(KD)
        ck[k] = 1
        dd = Ch.chebder(ck)
        DER[:len(dd), k] = dd
    DM = (DER @ F) / SCAL                                   # [KD, MN]
    return xn, DM.T.astype(np.float32)                      # dct1 [MN, KD]


_XN, _DCT1 = _host_consts()


def _build():
    nc = bacc.Bacc(debug=False, num_devices=NCORES)

    seq_sl = nc.dram_tensor("seq_sl", [B, C, LC], F32, kind="ExternalInput")
    expv = nc.dram_tensor("expv", [B, G], F32, kind="ExternalInput")
    smalls = nc.dram_tensor("smalls", [128, SM_NCOL], F32, kind="ExternalInput")
    rowsv = nc.dram_tensor("rowsv", [1, RW_NCOL], F32, kind="ExternalInput")
    w1a = nc.dram_tensor("w1a", [C, 4 * C], FP16, kind="ExternalInput")   # W1.T
    w2a = nc.dram_tensor("w2a", [4 * C, C], FP16, kind="ExternalInput")   # W2.T
    woa = nc.dram_tensor("woa", [C, C], FP16, kind="ExternalInput")       # Wo'.T
    out_sl = nc.dram_tensor("out_sl", [B, C, LC], F32, kind="ExternalOutput")
    dbg = {}
    if DBG:
        dbg["tt_sb"] = nc.dram_tensor("d_ttsb", [8, T], F32, kind="ExternalOutput")
        dbg["tt"] = nc.dram_tensor("d_tt", [128, FP], F32, kind="ExternalOutput")
        dbg["lnz"] = nc.dram_tensor("d_lnz", [MN, B], F32, kind="ExternalOutput")
        dbg["cbb"] = nc.dram_tensor("d_cbb", [128, KD], F32, kind="ExternalOutput")
        dbg["wp"] = nc.dram_tensor("d_wp", [128, FP], F32, kind="ExternalOutput")
        dbg["wH"] = nc.dram_tensor("d_wH", [H, T], F32, kind="ExternalOutput")
        dbg["y"] = nc.dram_tensor("d_y", [KC, 128, T], FP16, kind="ExternalOutput")
        dbg["x"] = nc.dram_tensor("d_x", [KC, 128, T], FP16, kind="ExternalOutput")
        dbg["h0"] = nc.dram_tensor("d_h0", [128, T], FP16, kind="ExternalOutput")
        dbg["y2"] = nc.dram_tensor("d_y2", [KC, 128, T], FP16, kind="ExternalOutput")
        dbg["rstd1"] = nc.dram_tensor("d_rstd1", [1, T], F32, kind="ExternalOutput")

    with tile.TileContext(nc) as tc, ExitStack() as ctx:
        p_w = ctx.enter_context(tc.tile_pool(name="w", bufs=1))
        p_act = ctx.enter_context(tc.tile_pool(name="act", bufs=1))
        p_sm = ctx.enter_context(tc.tile_pool(name="sm", bufs=1))
        ps_mm = ctx.enter_context(tc.tile_pool(name="psmm", bufs=3, space="PSUM"))
        ps_xa = ctx.enter_context(tc.tile_pool(name="psxa", bufs=2, space="PSUM"))
        ps_st = ctx.enter_context(tc.tile_pool(name="psst", bufs=1, space="PSUM"))
        ps_ck = ctx.enter_context(tc.tile_pool(name="psck", bufs=1, space="PSUM"))

        # ---- tiny on-chip constants (no DMA) -----------------------------
        wtile_f = p_sm.tile([128, T], F32, tag="warmf")
        nc.vector.memset(wtile_f[:], 0.0)
        wtile = p_sm.tile([128, T], F32R, tag="warm")
        nc.vector.tensor_copy(wtile[:], wtile_f[:])
        onesk = p_sm.tile([128, 1], FP16, tag="onesk")
        nc.vector.memset(onesk[:], 1.0 / C)
        eps_col = p_sm.tile([1, 1], F32, tag="epsc")
        nc.vector.memset(eps_col[:], EPS)

        # ---- DMA loads: 5 independent queue rows -------------------------
        sm = p_sm.tile([128, SM_NCOL], F32, tag="sm")
        nc.scalar.dma_start(sm[:], smalls[:])
        eb = p_act.tile([MN, B * G], F32, tag="eb")
        for b in range(B):
            nc.gpsimd.dma_start(eb[0:MN, b * G:(b + 1) * G],
                                expv[b, :][None, :].to_broadcast((MN, G)))

        rows = p_sm.tile([1, RW_NCOL], F32, tag="rows")
        nc.scalar.dma_start(rows[:], rowsv[:])
        xs = p_w.tile([128, KC, B, LC], F32R, tag="xs")
        for b in range(B):
            nc.sync.dma_start(xs[:, :, b, :],
                              seq_sl[b].rearrange("(kt p) l -> p kt l", p=128).bitcast(F32R))
        w1s = p_w.tile([128, KC, 4 * C], FP16, tag="w1")
        nc.sync.dma_start(w1s[:], w1a.rearrange("(kt p) m -> p kt m", p=128))

        w2s = p_w.tile([128, KH, C], FP16, tag="w2")
        nc.gpsimd.dma_start(w2s[:], w2a.rearrange("(kh p) m -> p kh m", p=128))
        wos = p_w.tile([128, KC, C], FP16, tag="wo")
        nc.gpsimd.dma_start(wos[:], woa.rearrange("(kt p) m -> p kt m", p=128))

        # ---- rounded f32r views of small matmul operands -----------------
        m4r = p_sm.tile([128, KC * 8], F32R, tag="m4r")
        nc.vector.tensor_copy(m4r[:], sm[:, SM_M:SM_M + KC * 8])
        rowsr = p_sm.tile([1, RW_NCOL], F32R, tag="rowsr")
        nc.vector.tensor_copy(rowsr[:], rows[:])
        uvhr = p_sm.tile([H, C], F32R, tag="uvhr")
        nc.vector.tensor_copy(uvhr[:], sm[0:H, SM_UVH:SM_UVH + C])

        # ---- PE warm-up while DMAs land ----------------------------------
        for i in range(NWARM):
            pw = ps_xa.tile([128, T], F32, tag="xa", name=f"warm{i}")
            nc.tensor.matmul(pw[0:8, :], wtile[:, 0:8], wtile[:], start=True, stop=True)

        # ---- a = x_seq @ M' + a0'  (pre-scaled to t units) ---------------
        pa = ps_ck.tile([8, T], F32, tag="ck", name="pa")
        for kt in range(KC):
            nc.tensor.matmul(pa[:], m4r[:, kt * 8:(kt + 1) * 8],
                             xs[:, kt, :, :],
                             start=(kt == 0), stop=False)
        nc.tensor.matmul(pa[:], rowsr[0:1, RW_A0R:RW_A0R + 8],
                         rowsr[0:1, RW_ONE:RW_ONE + T], start=False, stop=True)
        tt_sb = p_sm.tile([8, T], F32, tag="tts")
        nc.scalar.copy(tt_sb[:], pa[:])

        # ---- logZ sampling at 32 nodes, both batches ---------------------
        z2 = p_sm.tile([MN, B], F32, tag="z2")
        for b in range(B):
            pn = p_act.tile([MN, G], F32, tag="pn", bufs=2, name=f"pn{b}")
            nc.scalar.activation(pn[:], eb[:, b * G:(b + 1) * G], AF.Exp,
                                 scale=sm[0:MN, SM_XN:SM_XN + 1],
                                 accum_out=z2[:, b:b + 1])
        lnz = p_sm.tile([MN, B], F32, tag="lnz")
        nc.scalar.activation(lnz[:], z2[:], AF.Ln)
        sqpre = p_sm.tile([1, 1], F32, tag="sqpre")
        nc.scalar.activation(sqpre[:], eps_col[:], AF.Sqrt, bias=eps_col[:])

        if DBG:
            nc.gpsimd.dma_start(dbg["tt_sb"][:], tt_sb[:])
            nc.gpsimd.dma_start(dbg["lnz"][:], lnz[:])
        # repack to [128, 32], p = b*64 + h*8 + lhi, free = llo (l=lhi*32+llo)
        # (scalar-queue order: after the exp/lnz chain so it doesn't stall it)
        tt = p_sm.tile([128, FP], F32, tag="tt")
        for b in range(B):
            nc.scalar.dma_start(tt[b * 64:(b + 1) * 64, :],
                                tt_sb[:, b * LC:(b + 1) * LC])
        nc.vector.tensor_scalar_max(tt[:], tt[:], -1.0)
        nc.vector.tensor_scalar_min(tt[:], tt[:], 1.0)

        # coeffs: ck2 = dct1.T @ lnz [KD, B]; transpose; broadcast to [128, KD]
        ck2_ps = ps_ck.tile([KD, B], F32, tag="ck")
        nc.tensor.matmul(ck2_ps[:], sm[0:MN, SM_DCT:SM_DCT + KD], lnz[:],
                         start=True, stop=True)
        ck2_sb = p_sm.tile([KD, B], F32, tag="ck2s")
        nc.scalar.copy(ck2_sb[:], ck2_ps[:])
        ckT_ps = ps_ck.tile([B, KD], F32, tag="ck", name="ckT_ps")
        nc.tensor.transpose(ckT_ps[:], ck2_sb[:], sm[0:KD, SM_ID:SM_ID + KD])
        ckT_sb = p_sm.tile([B, KD], F32, tag="ckTs")
        nc.scalar.copy(ckT_sb[:], ckT_ps[:])
        cbb_ps = ps_ck.tile([128, KD], F32, tag="ck", name="cbb_ps")
        nc.tensor.matmul(cbb_ps[:], sm[0:B, SM_SEL:SM_SEL + 128], ckT_sb[:],
                         start=True, stop=True)
        cbb = p_sm.tile([128, KD], F32, tag="cbbs")
        nc.scalar.copy(cbb[:], cbb_ps[:])

        if DBG:
            nc.gpsimd.dma_start(dbg["tt"][:], tt[:])
            nc.gpsimd.dma_start(dbg["cbb"][:], cbb[:])
        # ---- Chebyshev T_k recurrence (vector) ---------------------------
        tt2 = p_sm.tile([128, FP], F32, tag="tt2")
        nc.vector.tensor_add(tt2[:], tt[:], tt[:])
        t_tiles = [None, tt]
        for k in range(2, KD):
            tk = p_sm.tile([128, FP], F32, tag=f"t{k}", name=f"t{k}")
            nc.vector.tensor_mul(tk[:], tt2[:], t_tiles[k - 1][:])
            if k == 2:
                nc.vector.tensor_scalar_sub(tk[:], tk[:], 1.0)   # T0 = 1
            else:
                nc.vector.tensor_sub(tk[:], tk[:], t_tiles[k - 2][:])
            t_tiles.append(tk)
            if k in (5, 8, 11, 14):      # PE keep-warm trickle
                tkr = p_sm.tile([128, 8], F32R, tag="tkr", name=f"tkr{k}")
                nc.gpsimd.tensor_copy(tkr[:], tk[:, 0:8])
                pw = ps_xa.tile([128, T], F32, tag="xa", name=f"trk{k}")
                nc.tensor.matmul(pw[0:8, :], tkr[:], wtile[:], start=True, stop=True)
        # t0 term is a constant: handled in the k=1 seed below.

        # ---- contraction sum_k c_k T_k (vector) --------------------------
        accA = p_sm.tile([128, FP], F32, tag="accA")
        accB = p_sm.tile([128, FP], F32, tag="accB")
        nc.vector.tensor_scalar(accA[:], tt[:], cbb[:, 1:2], cbb[:, 0:1],
                                op0=OP.mult, op1=OP.add)
        wp_t = p_sm.tile([128, FP], F32, tag="wp", name="wp")
        cur, nxt = accA, accB
        for k in range(2, KD):
            dst = wp_t if k == KD - 1 else nxt
            nc.vector.scalar_tensor_tensor(
                out=dst[:], in0=t_tiles[k][:], scalar=cbb[:, k:k + 1],
                in1=cur[:], op0=OP.mult, op1=OP.add)
            cur, nxt = dst, cur
        w_pack = cur

        if DBG:
            nc.gpsimd.dma_start(dbg["wp"][:], w_pack[:])
        def trickle(dep, nm):
            tkr = p_sm.tile([128, 8], F32R, tag="tkr", name=f"tkr{nm}")
            nc.gpsimd.tensor_copy(tkr[:], dep[:, 0:8])
            pw = ps_xa.tile([128, T], F32, tag="xa", name=f"trw{nm}")
            nc.tensor.matmul(pw[0:8, :], tkr[:], wtile[:], start=True, stop=True)

        def ln_stats_tile(st2, y_tile, kt, ph):
            """Mean contribution inline; squares on gpsimd for a deferred pass."""
            stA, stB, sqs = st2
            nc.tensor.matmul(stA[:], onesk[:], y_tile[:],
                             start=(kt == 0), stop=(kt == KC - 1))
            sq = p_act.tile([128, T], FP16, tag="sq", bufs=4, name=f"sq{ph}{kt}")
            nc.scalar.activation(sq[:], y_tile[:], AF.Square)
            sqs.append(sq)

        def ln_stats_close(st2):
            stA, stB, sqs = st2
            for kt, sq in enumerate(sqs):
                nc.tensor.matmul(stB[:], onesk[:], sq[:],
                                 start=(kt == 0), stop=(kt == KC - 1))

        def ln_stats_open(ph):
            stA = ps_st.tile([1, T], F32, tag="stA", name=f"stA{ph}")
            stB = ps_st.tile([1, T], F32, tag="stB", name=f"stB{ph}")
            return stA, stB, []

        # ---- unpack w to [H, T] and apply: y = w*u_v + c_v + x_seq -------
        wH = p_sm.tile([H, T], F32R, tag="wH")
        for b in range(B):
            nc.scalar.dma_start(wH[:, b * LC:(b + 1) * LC],
                                w_pack[b * 64:(b + 1) * 64, :].bitcast(F32R))
        y_t = []
        st1 = ln_stats_open("a")
        for kt in range(KC):
            xa = ps_xa.tile([128, T], F32, tag="xa", name=f"xa{kt}")
            nc.tensor.matmul(xa[:], uvhr[:, kt * 128:(kt + 1) * 128],
                             wH[:], start=True, stop=True)
            yk = p_act.tile([128, T], FP16, tag="y", bufs=4, name=f"y{kt}")
            eng = nc.vector
            eng.scalar_tensor_tensor(
                out=yk[:], in0=xa[:], scalar=sm[:, SM_CV + kt:SM_CV + kt + 1],
                in1=xs[:, kt, :, :].bitcast(F32), op0=OP.add, op1=OP.add)
            y_t.append(yk)
            ln_stats_tile(st1, yk, kt, "a")
            if kt in (1, 3):
                trickle(yk, f"y{kt}")

        if DBG:
            nc.gpsimd.dma_start(dbg["wH"][:], wH[:].bitcast(F32))
            for kt in range(KC):
                nc.gpsimd.dma_start(dbg["y"][kt], y_t[kt][:])

        def ln_rows(st2, ph, want_mu=False, want_q=False):
            """mean/meansq -> (mu, std, rstd, q=mu*rstd) rows [1, T]."""
            stA, stB = st2[0], st2[1]
            musq = p_sm.tile([1, T], F32, tag="lnr", bufs=6, name=f"musq{ph}")
            nc.scalar.activation(musq[:], stA[:], AF.Square)
            var = p_sm.tile([1, T], F32, tag="lnr", bufs=6, name=f"var{ph}")
            nc.vector.tensor_sub(var[:], stB[:], musq[:])
            std = p_sm.tile([1, T], F32R, tag="lnr", bufs=6, name=f"std{ph}")
            nc.scalar.activation(std[:], var[:], AF.Sqrt, bias=eps_col[:])
            pwln = ps_xa.tile([128, T], F32, tag="xa", name=f"pwln{ph}")
            nc.tensor.matmul(pwln[0:8, :], rowsr[0:1, RW_ONE:RW_ONE + 8], std[:],
                             start=True, stop=True)
            rstd_f = p_sm.tile([1, T], F32, tag="rstdf", bufs=2, name=f"rstdf{ph}")
            nc.vector.reciprocal_approx_fast(rstd_f[:], std[:].bitcast(F32))
            rstd = p_sm.tile([1, T], F32R, tag="rstd", bufs=2, name=f"rstd{ph}")
            nc.vector.tensor_copy(rstd[:], rstd_f[:])
            mu = q = None
            if want_mu:
                mu = p_sm.tile([1, T], F32R, tag="mu", bufs=2, name=f"mu{ph}")
                nc.vector.tensor_copy(mu[:], stA[:])
            if want_q:
                q = p_sm.tile([1, T], F32R, tag="q", bufs=2, name=f"q{ph}")
                nc.vector.tensor_mul(q[:], stA[:], rstd_f[:])
            return mu, std, rstd, q

        # ---- LN1 apply -> x ----------------------------------------------
        ln_stats_close(st1)
        _, _, rstd1, q1 = ln_rows(st1, "a", want_q=True)
        x_t = []
        for kt in range(KC):
            sl = slice(RW_G1 + kt * 128, RW_G1 + (kt + 1) * 128)
            pA = ps_mm.tile([128, T], F32, tag="mm", name=f"pA{kt}")
            nc.tensor.matmul(pA[:], rowsr[0:1, sl], rstd1[:],
                             start=True, stop=True)
            pB = ps_mm.tile([128, T], F32, tag="mm", name=f"pB{kt}")
            nc.tensor.matmul(pB[:], rowsr[0:1, sl], q1[:],
                             start=True, stop=True)
            eng = nc.vector
            tx = p_act.tile([128, T], F32, tag="tx", bufs=2, name=f"tx{kt}")
            eng.tensor_mul(tx[:], y_t[kt][:], pA[:])
            xo = p_act.tile([128, T], FP16, tag="x", bufs=4, name=f"x{kt}")
            eng.scalar_tensor_tensor(
                out=xo[:], in0=tx[:], scalar=sm[:, SM_BE1 + kt:SM_BE1 + kt + 1],
                in1=pB[:], op0=OP.add, op1=OP.subtract)
            x_t.append(xo)

        if DBG:
            nc.gpsimd.dma_start(dbg["rstd1"][:], rstd1[:].bitcast(F32))
            for kt in range(KC):
                nc.gpsimd.dma_start(dbg["x"][kt], x_t[kt][:])
        # ---- FFN1: h = relu(W1 @ x + b1) ---------------------------------
        h_t = []
        for mt in range(KH):
            pf = ps_mm.tile([128, T], F32, tag="mm", name=f"pf1{mt}")
            for kt in range(KC):
                nc.tensor.matmul(pf[:], w1s[:, kt, mt * 128:(mt + 1) * 128],
                                 x_t[kt][:], start=(kt == 0), stop=(kt == KC - 1))
            hm = p_act.tile([128, T], FP16, tag="h", bufs=KH, name=f"h{mt}")
            nc.scalar.activation(hm[:], pf[:], AF.Relu,
                                 bias=sm[:, SM_B1 + mt:SM_B1 + mt + 1])
            h_t.append(hm)

        if DBG:
            nc.gpsimd.dma_start(dbg["h0"][:], h_t[0][:])
        # ---- FFN2 + residual -> y2 ---------------------------------------
        y2_t = []
        st2 = ln_stats_open("b")
        for mt in range(KC):
            pf = ps_mm.tile([128, T], F32, tag="mm", name=f"pf2{mt}")
            for kh in range(KH):
                nc.tensor.matmul(pf[:], w2s[:, kh, mt * 128:(mt + 1) * 128],
                                 h_t[kh][:], start=(kh == 0), stop=(kh == KH - 1))
            y2 = p_act.tile([128, T], FP16, tag="y2", bufs=4, name=f"y2{mt}")
            eng = nc.vector
            eng.scalar_tensor_tensor(
                out=y2[:], in0=x_t[mt][:], scalar=sm[:, SM_B2 + mt:SM_B2 + mt + 1],
                in1=pf[:], op0=OP.add, op1=OP.add)
            y2_t.append(y2)
            ln_stats_tile(st2, y2, mt, "b")

        if DBG:
            for mt in range(KC):
                nc.gpsimd.dma_start(dbg["y2"][mt], y2_t[mt][:])
        # ---- LN2 folded into output projection ---------------------------
        ln_stats_close(st2)
        mu2, std2, rstd2, _ = ln_rows(st2, "b", want_mu=True)
        rb_ps = ps_xa.tile([128, T], F32, tag="xa", name="rb")
        nc.tensor.matmul(rb_ps[:], rowsr[0:1, RW_ONE:RW_ONE + 128],
                         rstd2[:], start=True, stop=True)
        rb_sb = p_sm.tile([128, T], F32, tag="rbs")
        nc.vector.tensor_copy(rb_sb[:], rb_ps[:])
        for mt in range(KC):
            po = ps_mm.tile([128, T], F32, tag="mm", name=f"po{mt}")
            for kt in range(KC):
                nc.tensor.matmul(po[:], wos[:, kt, mt * 128:(mt + 1) * 128],
                                 y2_t[kt][:], start=(kt == 0), stop=False)
            nc.tensor.matmul(po[:], rowsr[0:1, RW_NS2 + mt * 128:RW_NS2 + (mt + 1) * 128],
                             mu2[:], start=False, stop=False)
            nc.tensor.matmul(po[:], rowsr[0:1, RW_BO + mt * 128:RW_BO + (mt + 1) * 128],
                             std2[:], start=False, stop=True)
            om = p_act.tile([128, T], F32, tag="om", bufs=2, name=f"om{mt}")
            nc.vector.tensor_mul(om[:], po[:], rb_sb[:])
            seng = nc.sync if mt % 2 == 0 else nc.gpsimd
            seng.dma_start(out_sl[:, mt * 128:(mt + 1) * 128, :].rearrange("b c l -> c b l"),
                           om[:])

    nc.compile()
    return nc


def _host_pack(inputs):
    f32 = lambda x: np.asarray(x, dtype=np.float32)
    Wq, Wk, Wv, Wo = (f32(inputs[k]) for k in ("Wq", "Wk", "Wv", "Wo"))
    W1, W2 = f32(inputs["W1"]), f32(inputs["W2"])
    Wg = f32(inputs["Wg"])[:, 0]
    bg, bq, bv, b1, b2, bo = (f32(inputs[k]) for k in ("bg", "bq", "bv", "b1", "b2", "bo"))
    g1, be1, g2, be2 = (f32(inputs[k]) for k in ("g1", "beta1", "g2", "beta2"))

    u_k = Wk @ Wg
    u_v = Wv @ Wg
    c_v = Wv @ bg + bv
    M = np.zeros((C, H), np.float32)
    a0 = np.zeros(H, np.float32)
    for h in range(H):
        ukh = u_k[h * D:(h + 1) * D]
        M[:, h] = Wq[h * D:(h + 1) * D, :].T @ ukh
        a0[h] = bq[h * D:(h + 1) * D] @ ukh
    Mp = M * (SCALE / SCAL)
    a0p = a0 * (SCALE / SCAL)
    uvH = np.zeros((H, C), np.float32)
    for h in range(H):
        uvH[h, h * D:(h + 1) * D] = u_v[h * D:(h + 1) * D]
    Wop = Wo * g2[None, :]
    bop = bo + Wo @ be2
    s2 = Wop.sum(1)

    smalls = np.zeros((128, SM_NCOL), np.float32)
    for kt in range(KC):
        smalls[:, SM_M + kt * 8:SM_M + (kt + 1) * 8] = Mp[kt * 128:(kt + 1) * 128, :]
        smalls[:, SM_CV + kt] = c_v[kt * 128:(kt + 1) * 128]
        smalls[:, SM_B2 + kt] = b2[kt * 128:(kt + 1) * 128]
        smalls[:, SM_BE1 + kt] = be1[kt * 128:(kt + 1) * 128]
    smalls[0:H, SM_UVH:SM_UVH + C] = uvH
    smalls[0:MN, SM_DCT:SM_DCT + KD] = _DCT1
    smalls[0:KD, SM_ID:SM_ID + KD] = np.eye(KD, dtype=np.float32)
    for p in range(128):
        smalls[p // 64, SM_SEL + p] = 1.0
    smalls[0:MN, SM_XN] = _XN
    smalls[0:H, SM_A0] = a0p
    for mt in range(KH):
        smalls[:, SM_B1 + mt] = b1[mt * 128:(mt + 1) * 128]

    rowsv = np.zeros((1, RW_NCOL), np.float32)
    rowsv[0, RW_G1:RW_G1 + C] = g1
    rowsv[0, RW_NS2:RW_NS2 + C] = -s2
    rowsv[0, RW_ONE:RW_ONE + 512] = 1.0
    rowsv[0, RW_BO:RW_BO + C] = bop
    rowsv[0, RW_A0R:RW_A0R + H] = a0p

    f16t = lambda x: np.ascontiguousarray(x.T, dtype=np.float16)
    return {
        "expv": f32(inputs["exp"]),
        "smalls": smalls,
        "rowsv": rowsv,
        "w1a": f16t(W1),
        "w2a": f16t(W2),
        "woa": f16t(Wop),
    }


def kernel(**inputs):
    global _CACHE, LAST_RESULTS
    if _CACHE is None:
        _CACHE = _build()
    nc = _CACHE

    base = _host_pack(inputs)
    seq = np.asarray(inputs["seq"], dtype=np.float32)
    in_maps = []
    for c in range(NCORES):
        m = dict(base)
        m["seq_sl"] = np.ascontiguousarray(seq[:, :, c * LC:(c + 1) * LC])
        in_maps.append(m)

    res = run_bass_kernel_spmd(nc, in_maps, list(range(NCORES)), trace=TRACE,
                               **TRACE_KW)
    LAST_RESULTS = res
    out = np.empty((B, C, L), np.float32)
    for c in range(NCORES):
        out[:, :, c * LC:(c + 1) * LC] = res.results[c]["out_sl"]
    return out
